# revision 1
# baseline (speedup 1.0000x reference)
"""Trainium2 Bass kernel for nn_CorrClassLoss.

Reference computation (B=4, C=19, H=512, W=1024, N=5000, IGNORE=255):
  ref_class = argmax_c inputs_ref[b].reshape(C, H*W)      # flat W-major
  lin_ref   = 512*y_ref + x_ref    (NOTE: linearized with H, kept faithfully)
  lin_other = 512*y_other + x_other
  gathered  = ref_class[b, lin_ref]
  target[b, lin_other] = gathered  (scatter, last write wins; rest IGNORE)
  loss = mean over non-ignored pixels of -log_softmax(inputs_other)[b, target, px]

Since lin = 512*y + x with x,y in [0,512), only flat positions [0, 262144)
are ever touched, and at most N unique scatter destinations per batch
contribute to the loss:

  loss = -(1/cnt) * sum over unique dests d (last writer j, src s_j) of
         [ x_other[b, cls(s_j), d] - ln(sum_c exp(x_other[b, c, d])) ]
  cls(s) = argmax_c x_ref[b, c, s],  cnt = total unique dests.

Strategy (8 cores, data-parallel over (batch, half-of-correspondences)):
  Host does index-only math (dedup last-wins, split j by the pixel-half of
  s_j, pack padded gather-offset tables) and hands each core pixel-major
  transposed shards ref_t[px, c] / other_t[px, c] (a layout/sharding choice;
  all value compute happens on device).
  Device per core: [128,19]-run indirect gathers fetch the ref vector at s_j
  and the other vector at d_j; compact argmax one-hot (grouped max + is_ge);
  term1 = sum_j onehot_j . other_vec_j;  term2 = sum_j ln(sum_c exp
  (other_vec_j[c])) over valid j.  Output [1, 2] = (term1, term2).
  Host: loss = -(sum_cores term1 - term2) / cnt.
"""

import sys

if "/opt/trn_rl_repo" not in sys.path:
    sys.path.insert(0, "/opt/trn_rl_repo")

import numpy as np

B, C, H, W = 4, 19, 512, 1024
HW = H * W                 # 524288
NPIX = 262144              # touched flat range [0, 262144)
NPIX_H = NPIX // 2         # 131072 source pixels per core
N = 5000
NCORES = 8

P = 128                    # partitions

PAD_OFF = 1 << 28          # out-of-bounds offset => gather skipped, stays 0

CG_MAIN = 2688             # typical per-core capacity (21 columns)
CG_FALLBACK = 5120         # guaranteed upper bound (40 columns)

_programs = {}


def _build_program(cg):
    import concourse.bass as bass
    import concourse.bacc as bacc
    import concourse.mybir as mybir
    import concourse.tile as tile

    cgg = cg // P              # gather columns

    nc = bacc.Bacc("TRN2", target_bir_lowering=False, debug=False,
                   num_devices=NCORES)

    # pixel-major transposed shards: ref_t[px, c], other_t[px, c]
    ref_t = nc.dram_tensor("ref_t", [NPIX_H, C], mybir.dt.float32,
                           kind="ExternalInput")
    other_t = nc.dram_tensor("other_t", [NPIX, C], mybir.dt.float32,
                             kind="ExternalInput")
    # gather offsets: s_local*19 / d*19; element j at [j%P, j//P];
    # padded with PAD_OFF (gather skipped, row stays 0)
    s_off = nc.dram_tensor("s_off", [P, cgg], mybir.dt.int32,
                           kind="ExternalInput")
    d_off = nc.dram_tensor("d_off", [P, cgg], mybir.dt.int32,
                           kind="ExternalInput")
    out = nc.dram_tensor("out", [1, 2], mybir.dt.float32,
                         kind="ExternalOutput")

    ref_flat19 = ref_t.rearrange("p c -> (p c)")[:, None]
    other_flat19 = other_t.rearrange("p c -> (p c)")[:, None]

    with tile.TileContext(nc) as tc:
        with (
            tc.tile_pool(name="gb", bufs=1) as gb,
            tc.tile_pool(name="cons", bufs=1) as cons,
            tc.tile_pool(name="psum", bufs=1, space="PSUM") as psum,
        ):
            ones = cons.tile([P, 1], mybir.dt.float32)
            nc.gpsimd.memset(ones[:], 1.0)

            so = gb.tile([P, cgg], mybir.dt.int32)
            nc.sync.dma_start(out=so[:], in_=s_off[:, :])
            do = gb.tile([P, cgg], mybir.dt.int32)
            nc.sync.dma_start(out=do[:], in_=d_off[:, :])
            # pad mask depends only on so: compute early, off the tail
            pm = gb.tile([P, cgg], mybir.dt.float32)
            nc.vector.tensor_scalar(
                out=pm[:], in0=so[:], scalar1=NPIX_H * 19, scalar2=None,
                op0=mybir.AluOpType.is_lt,
            )
            # R needs no zero-init: pad rows' garbage is annihilated by
            # eq2*R2 (R2 pad rows ARE zeroed) and the pm mask on term2
            R = gb.tile([P, cgg * 19], mybir.dt.float32)
            R2 = gb.tile([P, cgg * 19], mybir.dt.float32)
            nc.vector.memset(R2[:], 0.0)
            # all ref gathers first: the argmax one-hot chain then overlaps
            # the other-vector gather stream
            for col in range(cgg):
                nc.gpsimd.indirect_dma_start(
                    out=R[:, col * 19:(col + 1) * 19],
                    out_offset=None,
                    in_=ref_flat19,
                    in_offset=bass.IndirectOffsetOnAxis(
                        ap=so[:, col:col + 1], axis=0),
                    bounds_check=NPIX_H * 19 - 1,
                    oob_is_err=False,
                )

            Rv = R[:].rearrange("p (g c) -> p g c", c=19)

            m2 = gb.tile([P, cgg], mybir.dt.float32)
            nc.vector.tensor_reduce(out=m2[:], in_=Rv,
                                    axis=mybir.AxisListType.X,
                                    op=mybir.AluOpType.max)
            eq2 = gb.tile([P, cgg * 19], mybir.dt.float32)
            eq2v = eq2[:].rearrange("p (g c) -> p g c", c=19)
            nc.vector.tensor_tensor(
                out=eq2v, in0=Rv,
                in1=m2[:, :, None].to_broadcast([P, cgg, 19]),
                op=mybir.AluOpType.is_ge,
            )

            # other-vector gathers in column halves; the dependent compute
            # for each half issues as soon as that half has landed
            t1g = gb.tile([P, cgg], mybir.dt.float32)
            e2 = gb.tile([P, cgg * 19], mybir.dt.float32)
            S2 = gb.tile([P, cgg], mybir.dt.float32)
            h0 = cgg // 2
            for lo, hi in ((0, h0), (h0, cgg)):
                for col in range(lo, hi):
                    nc.gpsimd.indirect_dma_start(
                        out=R2[:, col * 19:(col + 1) * 19],
                        out_offset=None,
                        in_=other_flat19,
                        in_offset=bass.IndirectOffsetOnAxis(
                            ap=do[:, col:col + 1], axis=0),
                        bounds_check=NPIX * 19 - 1,
                        oob_is_err=False,
                    )
                w = hi - lo
                if w == 0:
                    continue
                sl = slice(lo * 19, hi * 19)
                slg = slice(lo, hi)
                nc.vector.tensor_tensor(out=eq2[:, sl], in0=eq2[:, sl],
                                        in1=R2[:, sl],
                                        op=mybir.AluOpType.mult)
                nc.vector.tensor_reduce(
                    out=t1g[:, slg],
                    in_=eq2[:, sl].rearrange("p (g c) -> p g c", c=19),
                    axis=mybir.AxisListType.X,
                    op=mybir.AluOpType.add,
                )
                nc.scalar.activation(e2[:, sl], R2[:, sl],
                                     mybir.ActivationFunctionType.Exp)
                nc.vector.tensor_reduce(
                    out=S2[:, slg],
                    in_=e2[:, sl].rearrange("p (g c) -> p g c", c=19),
                    axis=mybir.AxisListType.X, op=mybir.AluOpType.add)
            L2 = gb.tile([P, cgg], mybir.dt.float32)
            nc.scalar.activation(L2[:], S2[:],
                                 mybir.ActivationFunctionType.Ln)
            nc.vector.tensor_tensor(out=L2[:], in0=L2[:], in1=pm[:],
                                    op=mybir.AluOpType.mult)

            # ---- combine ----------------------------------------------
            t1p = gb.tile([P, 1], mybir.dt.float32)
            nc.vector.tensor_reduce(out=t1p[:], in_=t1g[:],
                                    axis=mybir.AxisListType.X,
                                    op=mybir.AluOpType.add)
            t2p = gb.tile([P, 1], mybir.dt.float32)
            nc.vector.tensor_reduce(out=t2p[:], in_=L2[:],
                                    axis=mybir.AxisListType.X,
                                    op=mybir.AluOpType.add)
            pout = psum.tile([1, 2], mybir.dt.float32, space="PSUM")
            nc.tensor.matmul(out=pout[:, 0:1], lhsT=t1p[:], rhs=ones[:],
                             start=True, stop=True)
            nc.tensor.matmul(out=pout[:, 1:2], lhsT=t2p[:], rhs=ones[:],
                             start=True, stop=True)
            so_out = cons.tile([1, 2], mybir.dt.float32)
            nc.vector.tensor_copy(out=so_out[:], in_=pout[:])
            nc.sync.dma_start(out=out[:, :], in_=so_out[:])

    nc.finalize()
    return nc


def _get_program(cg):
    if cg not in _programs:
        _programs[cg] = _build_program(cg)
    return _programs[cg]


def _host_prep(inds_ref, inds_other):
    """Index-only host math: dedup scatter (last wins), partition per core."""
    ir = np.asarray(inds_ref).astype(np.int64)      # [B, 2, N]
    io = np.asarray(inds_other).astype(np.int64)
    valid = ((ir[:, 0] >= 0) & (ir[:, 0] < W) & (ir[:, 1] >= 0) & (ir[:, 1] < H)
             & (io[:, 0] >= 0) & (io[:, 0] < W) & (io[:, 1] >= 0)
             & (io[:, 1] < H))                       # [B, N]
    lin_ref = H * ir[:, 1] + ir[:, 0]                # [B, N]
    lin_other = H * io[:, 1] + io[:, 0]

    per_core = []
    count = 0
    need_fallback = False
    for b in range(B):
        v = valid[b]
        lo = lin_other[b][v]
        lr = np.clip(lin_ref[b][v], 0, HW - 1)
        # last-write-wins dedup on destinations
        u, first_rev = np.unique(lo[::-1], return_index=True)
        last_idx = len(lo) - 1 - first_rev
        d_arr = u.astype(np.int64)
        s_arr = lr[last_idx].astype(np.int64)
        count += len(u)
        for h in range(2):
            sel = (s_arr // NPIX_H) == h
            s_local = s_arr[sel] - h * NPIX_H
            d_sel = d_arr[sel]
            per_core.append({
                "b": b, "h": h,
                "s": s_local, "d": d_sel,
            })
    return per_core, count


def _pack_core(pc, cg):
    cgg = cg // P
    s_off = np.full((P, cgg), PAD_OFF, dtype=np.int32)
    d_off = np.full((P, cgg), PAD_OFF, dtype=np.int32)
    s, d = pc["s"], pc["d"]
    n = len(s)
    assert n <= cg
    jj = np.arange(n)
    s_off[jj % P, jj // P] = s * 19
    d_off[jj % P, jj // P] = d * 19
    return s_off, d_off


def _make_in_maps(inputs_ref, inputs_other, per_core, cg):
    ref_flat = inputs_ref.reshape(B, C, HW)
    other_flat = inputs_other.reshape(B, C, HW)
    # transposed shards; other_t shared by both cores of a batch pair
    other_cache = {}
    in_maps = []
    for pc in per_core:
        b, h = pc["b"], pc["h"]
        ref_td = np.ascontiguousarray(
            ref_flat[b, :, h * NPIX_H:(h + 1) * NPIX_H].T)
        if b not in other_cache:
            other_cache[b] = np.ascontiguousarray(other_flat[b, :, :NPIX].T)
        s_off, d_off = _pack_core(pc, cg)
        in_maps.append({
            "ref_t": ref_td,
            "other_t": other_cache[b],
            "s_off": s_off,
            "d_off": d_off,
        })
    return in_maps


def kernel(inputs_ref, inputs_other, inds_ref, inds_other, weights):
    from concourse.bass_utils import run_bass_kernel_spmd

    inputs_ref = np.asarray(inputs_ref, dtype=np.float32)
    inputs_other = np.asarray(inputs_other, dtype=np.float32)

    per_core, count = _host_prep(inds_ref, inds_other)
    # exact-fit capacity: compile (and cache) the program for the actual
    # worst-core correspondence count, rounded up to whole 128-columns
    max_n = max(len(pc["s"]) for pc in per_core)
    cg = max(128, -(-max_n // P) * P)
    nc = _get_program(cg)

    in_maps = _make_in_maps(inputs_ref, inputs_other, per_core, cg)
    res = run_bass_kernel_spmd(nc, in_maps, core_ids=list(range(NCORES)))
    total = 0.0
    for r in res.results:
        o = np.asarray(r["out"], dtype=np.float64)
        total += o[0, 0] - o[0, 1]
    loss = -total / max(count, 1)
    return np.float32(loss)



# revision 3
# speedup vs baseline: 1.6241x; 1.6241x over previous
"""Trainium2 Bass kernel for nn_CorrClassLoss.

Reference computation (B=4, C=19, H=512, W=1024, N=5000, IGNORE=255):
  ref_class = argmax_c inputs_ref[b].reshape(C, H*W)      # flat W-major
  lin_ref   = 512*y_ref + x_ref    (NOTE: linearized with H, kept faithfully)
  lin_other = 512*y_other + x_other
  gathered  = ref_class[b, lin_ref]
  target[b, lin_other] = gathered  (scatter, last write wins; rest IGNORE)
  loss = mean over non-ignored pixels of -log_softmax(inputs_other)[b, target, px]

Since lin = 512*y + x with x,y in [0,512), only flat positions [0, 262144)
are ever touched, and at most N unique scatter destinations per batch
contribute to the loss:

  loss = -(1/cnt) * sum over unique dests d (last writer j, src s_j) of
         [ x_other[b, cls(s_j), d] - ln(sum_c exp(x_other[b, c, d])) ]
  cls(s) = argmax_c x_ref[b, c, s],  cnt = total unique dests.

Strategy (8 cores, data-parallel over (batch, half-of-correspondences)):
  Host does index-only math (dedup last-wins, split j by the pixel-half of
  s_j, pack padded gather-offset tables) and hands each core pixel-major
  transposed shards ref_t[px, c] / other_t[px, c] (a layout/sharding choice;
  all value compute happens on device).
  Device per core: [128,19]-run indirect gathers fetch the ref vector at s_j
  and the other vector at d_j; compact argmax one-hot (grouped max + is_ge);
  term1 = sum_j onehot_j . other_vec_j;  term2 = sum_j ln(sum_c exp
  (other_vec_j[c])) over valid j.  Output [1, 2] = (term1, term2).
  Host: loss = -(sum_cores term1 - term2) / cnt.
"""

import sys

if "/opt/trn_rl_repo" not in sys.path:
    sys.path.insert(0, "/opt/trn_rl_repo")

import numpy as np

B, C, H, W = 4, 19, 512, 1024
HW = H * W                 # 524288
NPIX = 262144              # touched flat range [0, 262144)
NPIX_H = NPIX // 2         # 131072 source pixels per core
N = 5000
NCORES = 8

P = 128                    # partitions

PAD_OFF = 1 << 28          # out-of-bounds offset => gather skipped, stays 0

CG_MAIN = 2688             # typical per-core capacity (21 columns)
CG_FALLBACK = 5120         # guaranteed upper bound (40 columns)

_programs = {}


def _build_program(cg):
    import concourse.bass as bass
    import concourse.bacc as bacc
    import concourse.mybir as mybir
    import concourse.tile as tile

    cgg = cg // P              # gather columns

    nc = bacc.Bacc("TRN2", target_bir_lowering=False, debug=False,
                   num_devices=NCORES)

    # pixel-major transposed shards: ref_t[px, c], other_t[px, c]
    ref_t = nc.dram_tensor("ref_t", [NPIX_H, C], mybir.dt.float32,
                           kind="ExternalInput")
    other_t = nc.dram_tensor("other_t", [NPIX, C], mybir.dt.float32,
                             kind="ExternalInput")
    # gather offsets: s_local*19 / d*19; element j at [j%P, j//P];
    # padded with PAD_OFF (gather skipped, row stays 0)
    s_off = nc.dram_tensor("s_off", [P, cgg], mybir.dt.int32,
                           kind="ExternalInput")
    d_off = nc.dram_tensor("d_off", [P, cgg], mybir.dt.int32,
                           kind="ExternalInput")
    out = nc.dram_tensor("out", [1, 2], mybir.dt.float32,
                         kind="ExternalOutput")

    ref_flat19 = ref_t.rearrange("p c -> (p c)")[None, :]
    other_flat19 = other_t.rearrange("p c -> (p c)")[None, :]

    with tile.TileContext(nc) as tc:
        with (
            tc.tile_pool(name="gb", bufs=1) as gb,
            tc.tile_pool(name="cons", bufs=1) as cons,
            tc.tile_pool(name="psum", bufs=1, space="PSUM") as psum,
        ):
            ones = cons.tile([P, 1], mybir.dt.float32)
            nc.gpsimd.memset(ones[:], 1.0)

            so = gb.tile([P, cgg], mybir.dt.int32)
            nc.sync.dma_start(out=so[:], in_=s_off[:, :])
            do = gb.tile([P, cgg], mybir.dt.int32)
            nc.sync.dma_start(out=do[:], in_=d_off[:, :])
            # pad mask depends only on so: compute early, off the tail
            pm = gb.tile([P, cgg], mybir.dt.float32)
            nc.vector.tensor_scalar(
                out=pm[:], in0=so[:], scalar1=NPIX_H * 19, scalar2=None,
                op0=mybir.AluOpType.is_lt,
            )
            # R needs no zero-init: pad rows' garbage is annihilated by
            # eq2*R2 (R2 pad rows ARE zeroed) and the pm mask on term2
            R = gb.tile([P, cgg * 19], mybir.dt.float32)
            R2 = gb.tile([P, cgg * 19], mybir.dt.float32)
            nc.vector.memset(R2[:], 0.0)
            # all ref gathers first: the argmax one-hot chain then overlaps
            # the other-vector gather stream
            for col in range(cgg):
                nc.gpsimd.indirect_dma_start(
                    out=R[:, col * 19:(col + 1) * 19],
                    out_offset=None,
                    in_=ref_flat19,
                    in_offset=bass.IndirectOffsetOnAxis(
                        ap=so[:, col:col + 1], axis=1),
                    bounds_check=NPIX_H * 19 - 1,
                    oob_is_err=False,
                )

            Rv = R[:].rearrange("p (g c) -> p g c", c=19)

            m2 = gb.tile([P, cgg], mybir.dt.float32)
            nc.vector.tensor_reduce(out=m2[:], in_=Rv,
                                    axis=mybir.AxisListType.X,
                                    op=mybir.AluOpType.max)
            eq2 = gb.tile([P, cgg * 19], mybir.dt.float32)
            eq2v = eq2[:].rearrange("p (g c) -> p g c", c=19)
            nc.vector.tensor_tensor(
                out=eq2v, in0=Rv,
                in1=m2[:, :, None].to_broadcast([P, cgg, 19]),
                op=mybir.AluOpType.is_ge,
            )

            # other-vector gathers in column halves; the dependent compute
            # for each half issues as soon as that half has landed
            t1g = gb.tile([P, cgg], mybir.dt.float32)
            e2 = gb.tile([P, cgg * 19], mybir.dt.float32)
            S2 = gb.tile([P, cgg], mybir.dt.float32)
            h0 = cgg // 2
            for lo, hi in ((0, h0), (h0, cgg)):
                for col in range(lo, hi):
                    nc.gpsimd.indirect_dma_start(
                        out=R2[:, col * 19:(col + 1) * 19],
                        out_offset=None,
                        in_=other_flat19,
                        in_offset=bass.IndirectOffsetOnAxis(
                            ap=do[:, col:col + 1], axis=1),
                        bounds_check=NPIX * 19 - 1,
                        oob_is_err=False,
                    )
                w = hi - lo
                if w == 0:
                    continue
                sl = slice(lo * 19, hi * 19)
                slg = slice(lo, hi)
                nc.vector.tensor_tensor(out=eq2[:, sl], in0=eq2[:, sl],
                                        in1=R2[:, sl],
                                        op=mybir.AluOpType.mult)
                nc.vector.tensor_reduce(
                    out=t1g[:, slg],
                    in_=eq2[:, sl].rearrange("p (g c) -> p g c", c=19),
                    axis=mybir.AxisListType.X,
                    op=mybir.AluOpType.add,
                )
                nc.scalar.activation(e2[:, sl], R2[:, sl],
                                     mybir.ActivationFunctionType.Exp)
                nc.vector.tensor_reduce(
                    out=S2[:, slg],
                    in_=e2[:, sl].rearrange("p (g c) -> p g c", c=19),
                    axis=mybir.AxisListType.X, op=mybir.AluOpType.add)
            L2 = gb.tile([P, cgg], mybir.dt.float32)
            nc.scalar.activation(L2[:], S2[:],
                                 mybir.ActivationFunctionType.Ln)
            nc.vector.tensor_tensor(out=L2[:], in0=L2[:], in1=pm[:],
                                    op=mybir.AluOpType.mult)

            # ---- combine ----------------------------------------------
            t1p = gb.tile([P, 1], mybir.dt.float32)
            nc.vector.tensor_reduce(out=t1p[:], in_=t1g[:],
                                    axis=mybir.AxisListType.X,
                                    op=mybir.AluOpType.add)
            t2p = gb.tile([P, 1], mybir.dt.float32)
            nc.vector.tensor_reduce(out=t2p[:], in_=L2[:],
                                    axis=mybir.AxisListType.X,
                                    op=mybir.AluOpType.add)
            pout = psum.tile([1, 2], mybir.dt.float32, space="PSUM")
            nc.tensor.matmul(out=pout[:, 0:1], lhsT=t1p[:], rhs=ones[:],
                             start=True, stop=True)
            nc.tensor.matmul(out=pout[:, 1:2], lhsT=t2p[:], rhs=ones[:],
                             start=True, stop=True)
            so_out = cons.tile([1, 2], mybir.dt.float32)
            nc.vector.tensor_copy(out=so_out[:], in_=pout[:])
            nc.sync.dma_start(out=out[:, :], in_=so_out[:])

    nc.finalize()
    return nc


def _get_program(cg):
    if cg not in _programs:
        _programs[cg] = _build_program(cg)
    return _programs[cg]


def _host_prep(inds_ref, inds_other):
    """Index-only host math: dedup scatter (last wins), partition per core."""
    ir = np.asarray(inds_ref).astype(np.int64)      # [B, 2, N]
    io = np.asarray(inds_other).astype(np.int64)
    valid = ((ir[:, 0] >= 0) & (ir[:, 0] < W) & (ir[:, 1] >= 0) & (ir[:, 1] < H)
             & (io[:, 0] >= 0) & (io[:, 0] < W) & (io[:, 1] >= 0)
             & (io[:, 1] < H))                       # [B, N]
    lin_ref = H * ir[:, 1] + ir[:, 0]                # [B, N]
    lin_other = H * io[:, 1] + io[:, 0]

    per_core = []
    count = 0
    need_fallback = False
    for b in range(B):
        v = valid[b]
        lo = lin_other[b][v]
        lr = np.clip(lin_ref[b][v], 0, HW - 1)
        # last-write-wins dedup on destinations
        u, first_rev = np.unique(lo[::-1], return_index=True)
        last_idx = len(lo) - 1 - first_rev
        d_arr = u.astype(np.int64)
        s_arr = lr[last_idx].astype(np.int64)
        count += len(u)
        for h in range(2):
            sel = (s_arr // NPIX_H) == h
            s_local = s_arr[sel] - h * NPIX_H
            d_sel = d_arr[sel]
            per_core.append({
                "b": b, "h": h,
                "s": s_local, "d": d_sel,
            })
    return per_core, count


def _pack_core(pc, cg):
    cgg = cg // P
    s_off = np.full((P, cgg), PAD_OFF, dtype=np.int32)
    d_off = np.full((P, cgg), PAD_OFF, dtype=np.int32)
    s, d = pc["s"], pc["d"]
    n = len(s)
    assert n <= cg
    jj = np.arange(n)
    s_off[jj % P, jj // P] = s * 19
    d_off[jj % P, jj // P] = d * 19
    return s_off, d_off


def _make_in_maps(inputs_ref, inputs_other, per_core, cg):
    ref_flat = inputs_ref.reshape(B, C, HW)
    other_flat = inputs_other.reshape(B, C, HW)
    # transposed shards; other_t shared by both cores of a batch pair
    other_cache = {}
    in_maps = []
    for pc in per_core:
        b, h = pc["b"], pc["h"]
        ref_td = np.ascontiguousarray(
            ref_flat[b, :, h * NPIX_H:(h + 1) * NPIX_H].T)
        if b not in other_cache:
            other_cache[b] = np.ascontiguousarray(other_flat[b, :, :NPIX].T)
        s_off, d_off = _pack_core(pc, cg)
        in_maps.append({
            "ref_t": ref_td,
            "other_t": other_cache[b],
            "s_off": s_off,
            "d_off": d_off,
        })
    return in_maps


def kernel(inputs_ref, inputs_other, inds_ref, inds_other, weights):
    from concourse.bass_utils import run_bass_kernel_spmd

    inputs_ref = np.asarray(inputs_ref, dtype=np.float32)
    inputs_other = np.asarray(inputs_other, dtype=np.float32)

    per_core, count = _host_prep(inds_ref, inds_other)
    # exact-fit capacity: compile (and cache) the program for the actual
    # worst-core correspondence count, rounded up to whole 128-columns
    max_n = max(len(pc["s"]) for pc in per_core)
    cg = max(128, -(-max_n // P) * P)
    nc = _get_program(cg)

    in_maps = _make_in_maps(inputs_ref, inputs_other, per_core, cg)
    res = run_bass_kernel_spmd(nc, in_maps, core_ids=list(range(NCORES)))
    total = 0.0
    for r in res.results:
        o = np.asarray(r["out"], dtype=np.float64)
        total += o[0, 0] - o[0, 1]
    loss = -total / max(count, 1)
    return np.float32(loss)



# revision 4
# speedup vs baseline: 2.1335x; 1.3136x over previous
"""Trainium2 Bass kernel for nn_CorrClassLoss.

Reference computation (B=4, C=19, H=512, W=1024, N=5000, IGNORE=255):
  ref_class = argmax_c inputs_ref[b].reshape(C, H*W)      # flat W-major
  lin_ref   = 512*y_ref + x_ref    (NOTE: linearized with H, kept faithfully)
  lin_other = 512*y_other + x_other
  gathered  = ref_class[b, lin_ref]
  target[b, lin_other] = gathered  (scatter, last write wins; rest IGNORE)
  loss = mean over non-ignored pixels of -log_softmax(inputs_other)[b, target, px]

Only flat positions [0, 262144) are touched; at most N unique scatter dests
per batch contribute:

  loss = -(1/cnt) * sum over unique dests d (last writer j, src s_j) of
         [ x_other[b, cls(s_j), d] - ln(sum_c exp(x_other[b, c, d])) ]
  cls(s) = argmax_c x_ref[b, c, s],  cnt = total unique dests.

Strategy (8 cores, data-parallel over (batch, half-of-sources)):
  Host does index-only math (dedup last-wins, split by source half, sort by
  source 32K-row window, pack idx/offset tables) plus pure relayout of the
  image data (pixel-major transpose, 64-slot padded rows for the ref table).
  Device per core:
    - 4x InstDMAGatherAnt fetch ref rows (64-f32 slots, int16 window-local
      idx) into an s-sorted slot space [128, Gs] (slot j = [j%128, j//128]).
    - Gs indirect DMAs fetch the 19-f32 other vectors per slot column.
    - argmax one-hot via strided 19-of-64 APs (max + is_ge), then
      term1 = sum onehot . other_vec, term2 = sum ln(sum_c exp(other_vec)).
  Host: loss = -(sum_cores term1 - term2) / cnt.
"""

import sys

if "/opt/trn_rl_repo" not in sys.path:
    sys.path.insert(0, "/opt/trn_rl_repo")

import numpy as np

B, C, H, W = 4, 19, 512, 1024
HW = H * W                 # 524288
NPIX = 262144              # touched flat range [0, 262144)
NPIX_H = NPIX // 2         # 131072 source pixels per core
N = 5000
NCORES = 8

P = 128                    # partitions
E64 = 64                   # f32 slots per ref-table row (256B, dma_gather unit)
W_ROWS = 32768             # rows per dma_gather window (int16 idx range)
NW_S = NPIX_H // W_ROWS    # 4 source windows per core

PAD_OFF = 1 << 28          # out-of-bounds offset => gather skipped, stays 0

_programs = {}


def _build_program(key):
    import concourse.bass as bass
    import concourse.bacc as bacc
    import concourse.mybir as mybir
    import concourse.tile as tile

    G = list(key)              # columns per source window
    Gs = sum(G)
    offs = np.concatenate([[0], np.cumsum(G)]).astype(int)

    nc = bacc.Bacc("TRN2", target_bir_lowering=False, debug=False,
                   num_devices=NCORES)

    ref64 = nc.dram_tensor("ref64", [NPIX_H, E64], mybir.dt.float32,
                           kind="ExternalInput")
    other_t = nc.dram_tensor("other_t", [NPIX, C], mybir.dt.float32,
                             kind="ExternalInput")
    s_idx = nc.dram_tensor("s_idx", [P, Gs * 8], mybir.dt.int16,
                           kind="ExternalInput")
    d_off = nc.dram_tensor("d_off", [P, Gs], mybir.dt.int32,
                           kind="ExternalInput")
    out = nc.dram_tensor("out", [1, 2], mybir.dt.float32,
                         kind="ExternalOutput")

    other_flat19 = other_t.rearrange("p c -> (p c)")[None, :]

    with tile.TileContext(nc) as tc:
        with (
            tc.tile_pool(name="gb", bufs=1) as gb,
            tc.tile_pool(name="cons", bufs=1) as cons,
            tc.tile_pool(name="psum", bufs=1, space="PSUM") as psum,
        ):
            ones = cons.tile([P, 1], mybir.dt.float32)
            nc.vector.memset(ones[:], 1.0)

            si = gb.tile([P, Gs * 8], mybir.dt.int16)
            nc.sync.dma_start(out=si[:], in_=s_idx[:, :])
            do = gb.tile([P, Gs], mybir.dt.int32)
            nc.sync.dma_start(out=do[:], in_=d_off[:, :])
            # pad mask: pad slots carry d_off = PAD_OFF
            pm = gb.tile([P, Gs], mybir.dt.float32)
            nc.vector.tensor_scalar(
                out=pm[:], in0=do[:], scalar1=NPIX * 19, scalar2=None,
                op0=mybir.AluOpType.is_lt,
            )

            REF = gb.tile([P, Gs * E64], mybir.dt.float32)
            R2 = gb.tile([P, Gs * 19], mybir.dt.float32)
            nc.vector.memset(R2[:], 0.0)

            # s-side: one gather per 32K-row source window
            for k in range(NW_S):
                if G[k] == 0:
                    continue
                nc.gpsimd.dma_gather(
                    out_ap=REF[:, offs[k] * E64:offs[k + 1] * E64].rearrange(
                        "p (g c) -> p g c", c=E64),
                    in_ap=ref64[k * W_ROWS:(k + 1) * W_ROWS, :],
                    idxs_ap=si[:, offs[k] * 8:offs[k + 1] * 8],
                    num_idxs=G[k] * P,
                    num_idxs_reg=G[k] * P,
                    elem_size=E64,
                )

            Rv19 = REF[:].rearrange("p (g c) -> p g c", c=E64)[:, :, 0:19]

            m2 = gb.tile([P, Gs], mybir.dt.float32)
            nc.vector.tensor_reduce(out=m2[:], in_=Rv19,
                                    axis=mybir.AxisListType.X,
                                    op=mybir.AluOpType.max)
            eq2 = gb.tile([P, Gs * 19], mybir.dt.float32)
            eq2v = eq2[:].rearrange("p (g c) -> p g c", c=19)
            nc.vector.tensor_tensor(
                out=eq2v, in0=Rv19,
                in1=m2[:, :, None].to_broadcast([P, Gs, 19]),
                op=mybir.AluOpType.is_ge,
            )

            # d-side: per-column indirect gathers of the 19-f32 other vectors;
            # compute for each half issues as soon as that half has landed
            t1g = gb.tile([P, Gs], mybir.dt.float32)
            e2 = gb.tile([P, Gs * 19], mybir.dt.float32)
            S2 = gb.tile([P, Gs], mybir.dt.float32)
            h0 = Gs // 2
            for lo, hi in ((0, h0), (h0, Gs)):
                for col in range(lo, hi):
                    nc.gpsimd.indirect_dma_start(
                        out=R2[:, col * 19:(col + 1) * 19],
                        out_offset=None,
                        in_=other_flat19,
                        in_offset=bass.IndirectOffsetOnAxis(
                            ap=do[:, col:col + 1], axis=1),
                        bounds_check=NPIX * 19 - 1,
                        oob_is_err=False,
                    )
                w = hi - lo
                if w == 0:
                    continue
                sl = slice(lo * 19, hi * 19)
                slg = slice(lo, hi)
                nc.vector.tensor_tensor(out=eq2[:, sl], in0=eq2[:, sl],
                                        in1=R2[:, sl],
                                        op=mybir.AluOpType.mult)
                nc.vector.tensor_reduce(
                    out=t1g[:, slg],
                    in_=eq2[:, sl].rearrange("p (g c) -> p g c", c=19),
                    axis=mybir.AxisListType.X,
                    op=mybir.AluOpType.add,
                )
                nc.scalar.activation(e2[:, sl], R2[:, sl],
                                     mybir.ActivationFunctionType.Exp)
                nc.vector.tensor_reduce(
                    out=S2[:, slg],
                    in_=e2[:, sl].rearrange("p (g c) -> p g c", c=19),
                    axis=mybir.AxisListType.X, op=mybir.AluOpType.add)
            L2 = gb.tile([P, Gs], mybir.dt.float32)
            nc.scalar.activation(L2[:], S2[:],
                                 mybir.ActivationFunctionType.Ln)
            nc.vector.tensor_tensor(out=L2[:], in0=L2[:], in1=pm[:],
                                    op=mybir.AluOpType.mult)

            # ---- combine ----------------------------------------------
            t1p = gb.tile([P, 1], mybir.dt.float32)
            nc.vector.tensor_reduce(out=t1p[:], in_=t1g[:],
                                    axis=mybir.AxisListType.X,
                                    op=mybir.AluOpType.add)
            t2p = gb.tile([P, 1], mybir.dt.float32)
            nc.vector.tensor_reduce(out=t2p[:], in_=L2[:],
                                    axis=mybir.AxisListType.X,
                                    op=mybir.AluOpType.add)
            pout = psum.tile([1, 2], mybir.dt.float32, space="PSUM")
            nc.tensor.matmul(out=pout[:, 0:1], lhsT=t1p[:], rhs=ones[:],
                             start=True, stop=True)
            nc.tensor.matmul(out=pout[:, 1:2], lhsT=t2p[:], rhs=ones[:],
                             start=True, stop=True)
            so_out = cons.tile([1, 2], mybir.dt.float32)
            nc.vector.tensor_copy(out=so_out[:], in_=pout[:])
            nc.sync.dma_start(out=out[:, :], in_=so_out[:])

    nc.finalize()
    return nc


def _get_program(key):
    if key not in _programs:
        _programs[key] = _build_program(key)
    return _programs[key]


def _host_prep(inds_ref, inds_other):
    """Index-only host math: dedup scatter (last wins), split per core,
    sort by source window."""
    ir = np.asarray(inds_ref).astype(np.int64)      # [B, 2, N]
    io = np.asarray(inds_other).astype(np.int64)
    valid = ((ir[:, 0] >= 0) & (ir[:, 0] < W) & (ir[:, 1] >= 0) & (ir[:, 1] < H)
             & (io[:, 0] >= 0) & (io[:, 0] < W) & (io[:, 1] >= 0)
             & (io[:, 1] < H))                       # [B, N]
    lin_ref = H * ir[:, 1] + ir[:, 0]                # [B, N]
    lin_other = H * io[:, 1] + io[:, 0]

    per_core = []
    count = 0
    for b in range(B):
        v = valid[b]
        lo = lin_other[b][v]
        lr = np.clip(lin_ref[b][v], 0, HW - 1)
        # last-write-wins dedup on destinations
        u, first_rev = np.unique(lo[::-1], return_index=True)
        d_arr = u.astype(np.int64)
        s_arr = lr[len(lo) - 1 - first_rev].astype(np.int64)
        count += len(u)
        for h in range(2):
            sel = (s_arr // NPIX_H) == h
            s_local = s_arr[sel] - h * NPIX_H
            d_sel = d_arr[sel]
            # sort by source window (stable keeps in-window order)
            kw = s_local // W_ROWS
            order = np.argsort(kw, kind='stable')
            s_srt = s_local[order]
            d_srt = d_sel[order]
            nk = np.bincount(kw, minlength=NW_S).astype(int)
            per_core.append({"s": s_srt, "d": d_srt, "nk": nk})
    return per_core, count


def _plan(per_core):
    nk_all = np.stack([pc["nk"] for pc in per_core])          # [8, NW_S]
    G = np.maximum(1, -(-nk_all.max(axis=0) // P))            # cols per window
    return tuple(int(g) for g in G)


def _pack_core(pc, key):
    G = np.asarray(key)
    Gs = int(G.sum())
    offs = np.concatenate([[0], np.cumsum(G)]).astype(int)
    s_idx16 = np.zeros((16, Gs * 8), dtype=np.int16)
    d_off = np.full((P, Gs), PAD_OFF, dtype=np.int32)
    pos = 0
    for k in range(NW_S):
        n = int(pc["nk"][k])
        s_w = pc["s"][pos:pos + n] - k * W_ROWS
        d_w = pc["d"][pos:pos + n]
        pos += n
        jj = np.arange(n)
        s_idx16[jj % 16, offs[k] * 8 + jj // 16] = s_w.astype(np.int16)
        d_off[jj % P, offs[k] + jj // P] = (d_w * 19).astype(np.int32)
    s_idx = np.tile(s_idx16, (8, 1))
    return s_idx, d_off


def _make_in_maps(inputs_ref, inputs_other, per_core, key):
    ref_flat = inputs_ref.reshape(B, C, HW)
    other_flat = inputs_other.reshape(B, C, HW)
    other_cache = {}
    in_maps = []
    for ci, pc in enumerate(per_core):
        b, h = ci // 2, ci % 2
        ref64 = np.empty((NPIX_H, E64), dtype=np.float32)
        ref64[:, :C] = ref_flat[b, :, h * NPIX_H:(h + 1) * NPIX_H].T
        if b not in other_cache:
            other_cache[b] = np.ascontiguousarray(other_flat[b, :, :NPIX].T)
        s_idx, d_off = _pack_core(pc, key)
        in_maps.append({
            "ref64": ref64,
            "other_t": other_cache[b],
            "s_idx": s_idx,
            "d_off": d_off,
        })
    return in_maps


def kernel(inputs_ref, inputs_other, inds_ref, inds_other, weights):
    from concourse.bass_utils import run_bass_kernel_spmd

    inputs_ref = np.asarray(inputs_ref, dtype=np.float32)
    inputs_other = np.asarray(inputs_other, dtype=np.float32)

    per_core, count = _host_prep(inds_ref, inds_other)
    key = _plan(per_core)
    nc = _get_program(key)

    in_maps = _make_in_maps(inputs_ref, inputs_other, per_core, key)
    res = run_bass_kernel_spmd(nc, in_maps, core_ids=list(range(NCORES)))
    total = 0.0
    for r in res.results:
        o = np.asarray(r["out"], dtype=np.float64)
        total += o[0, 0] - o[0, 1]
    loss = -total / max(count, 1)
    return np.float32(loss)


# revision 6
# speedup vs baseline: 2.7824x; 1.3042x over previous
"""Trainium2 Bass kernel for nn_CorrClassLoss.

Reference computation (B=4, C=19, H=512, W=1024, N=5000, IGNORE=255):
  ref_class = argmax_c inputs_ref[b].reshape(C, H*W)      # flat W-major
  lin_ref   = 512*y_ref + x_ref    (NOTE: linearized with H, kept faithfully)
  lin_other = 512*y_other + x_other
  gathered  = ref_class[b, lin_ref]
  target[b, lin_other] = gathered  (scatter, last write wins; rest IGNORE)
  loss = mean over non-ignored pixels of -log_softmax(inputs_other)[b, target, px]

Only flat positions [0, 262144) are touched; at most N unique scatter dests
per batch contribute:

  loss = -(1/cnt) * sum over unique dests d (last writer j, src s_j) of
         [ x_other[b, cls(s_j), d] - ln(sum_c exp(x_other[b, c, d])) ]
  cls(s) = argmax_c x_ref[b, c, s],  cnt = total unique dests.

Strategy (8 cores, data-parallel over (batch, half-of-sources)). Host does
index-only math (dedup last-wins, core split, window sort, idx packing) plus
pure relayout of image data (pixel-major transpose into 64-slot rows).
Device per core, all value compute on device:
  - 8x InstDMAGatherAnt fetch other rows (64-f32 slots, int16 window-local
    idx) into a d-sorted slot space [128, Gd] (slot j = [j%128, j//128]);
    logsumexp term ln(sum_c exp(.)) reduced there (masked by pm_d).
  - the fetched other rows are dumped to a DRAM scratch (one strided DMA)
    and regathered (1x InstDMAGatherAnt) into the s-sorted slot space
    [128, Gs] built by 4x InstDMAGatherAnt of ref rows, where the argmax
    one-hot (max + is_ge on strided 19-of-64 APs) pairs with them:
    term1 = sum onehot . other_vec (masked by pm_s).
  Output [1, 2] = (term1_sum, term2_sum);
  host: loss = -(sum_cores term1 - term2) / cnt.
"""

import sys

if "/opt/trn_rl_repo" not in sys.path:
    sys.path.insert(0, "/opt/trn_rl_repo")

import numpy as np

B, C, H, W = 4, 19, 512, 1024
HW = H * W                 # 524288
NPIX = 262144              # touched flat range [0, 262144)
NPIX_H = NPIX // 2         # 131072 source pixels per core
N = 5000
NCORES = 8

P = 128                    # partitions
E64 = 64                   # f32 slots per table row (256B, dma_gather unit)
W_ROWS = 32768             # rows per dma_gather window (int16 idx range)
NW_S = NPIX_H // W_ROWS    # 4 source windows per core
NW_D = NPIX // W_ROWS      # 8 dest windows per core

_programs = {}


def _build_program(key):
    import concourse.bass as bass
    import concourse.bacc as bacc
    import concourse.mybir as mybir
    import concourse.tile as tile

    GS = list(key[0])          # columns per source window
    GD = list(key[1])          # columns per dest window
    Gs, Gd = sum(GS), sum(GD)
    offs = np.concatenate([[0], np.cumsum(GS)]).astype(int)
    offd = np.concatenate([[0], np.cumsum(GD)]).astype(int)

    nc = bacc.Bacc("TRN2", target_bir_lowering=False, debug=False,
                   num_devices=NCORES)

    ref64 = nc.dram_tensor("ref64", [NPIX_H, E64], mybir.dt.float32,
                           kind="ExternalInput")
    other64 = nc.dram_tensor("other64", [NPIX, E64], mybir.dt.float32,
                             kind="ExternalInput")
    # idx streams (int16, 16-wrapped, replicated x8): [d | s | r]
    idx16 = nc.dram_tensor("idx16", [P, (Gd + 2 * Gs) * 8], mybir.dt.int16,
                           kind="ExternalInput")
    # valid masks: [pm_d | pm_s]
    pmio = nc.dram_tensor("pmio", [P, Gd + Gs], mybir.dt.float32,
                          kind="ExternalInput")
    scratch = nc.dram_tensor("scratch", [P * Gd, E64], mybir.dt.float32,
                             kind="Internal")
    out = nc.dram_tensor("out", [1, 2], mybir.dt.float32,
                         kind="ExternalOutput")

    with tile.TileContext(nc) as tc:
        with (
            tc.tile_pool(name="gb", bufs=1) as gb,
            tc.tile_pool(name="cons", bufs=1) as cons,
            tc.tile_pool(name="psum", bufs=1, space="PSUM") as psum,
        ):
            ones = cons.tile([P, 1], mybir.dt.float32)
            nc.vector.memset(ones[:], 1.0)

            ix = gb.tile([P, (Gd + 2 * Gs) * 8], mybir.dt.int16)
            nc.sync.dma_start(out=ix[:], in_=idx16[:, :])
            pm = gb.tile([P, Gd + Gs], mybir.dt.float32)
            nc.sync.dma_start(out=pm[:], in_=pmio[:, :])

            OTH = gb.tile([P, Gd * E64], mybir.dt.float32)
            REF = gb.tile([P, Gs * E64], mybir.dt.float32)
            R2S = gb.tile([P, Gs * E64], mybir.dt.float32)

            # d-side: one gather per 32K-row dest window (critical chain:
            # feeds the scratch dump + regather)
            for m in range(NW_D):
                if GD[m] == 0:
                    continue
                nc.gpsimd.dma_gather(
                    out_ap=OTH[:, offd[m] * E64:offd[m + 1] * E64].rearrange(
                        "p (g c) -> p g c", c=E64),
                    in_ap=other64[m * W_ROWS:(m + 1) * W_ROWS, :],
                    idxs_ap=ix[:, offd[m] * 8:offd[m + 1] * 8],
                    num_idxs=GD[m] * P,
                    num_idxs_reg=GD[m] * P,
                    elem_size=E64,
                )
            # s-side: ref rows into the s-sorted slot space (fills the Pool
            # gap while the dump completes)
            sbase = Gd * 8
            for k in range(NW_S):
                if GS[k] == 0:
                    continue
                nc.gpsimd.dma_gather(
                    out_ap=REF[:, offs[k] * E64:offs[k + 1] * E64].rearrange(
                        "p (g c) -> p g c", c=E64),
                    in_ap=ref64[k * W_ROWS:(k + 1) * W_ROWS, :],
                    idxs_ap=ix[:, sbase + offs[k] * 8:sbase + offs[k + 1] * 8],
                    num_idxs=GS[k] * P,
                    num_idxs_reg=GS[k] * P,
                    elem_size=E64,
                )

            OTHv = OTH[:].rearrange("p (g c) -> p g c", c=E64)[:, :, 0:19]
            # dump the useful 19-of-64 of each fetched other row to scratch
            # row (p*Gd + g); regather below routes them to s-slot order
            nc.sync.dma_start(
                out=scratch.rearrange("(p g) c -> p g c", g=Gd)[:, :, 0:19],
                in_=OTHv,
            )

            # term2 in d-space: ln(sum_c exp(other_vec)), masked
            e2 = gb.tile([P, Gd * 19], mybir.dt.float32)
            e2v = e2[:].rearrange("p (g c) -> p g c", c=19)
            nc.scalar.activation(e2v, OTHv, mybir.ActivationFunctionType.Exp)
            S2 = gb.tile([P, Gd], mybir.dt.float32)
            nc.vector.tensor_reduce(out=S2[:], in_=e2v,
                                    axis=mybir.AxisListType.X,
                                    op=mybir.AluOpType.add)
            L2 = gb.tile([P, Gd], mybir.dt.float32)
            nc.scalar.activation(L2[:], S2[:], mybir.ActivationFunctionType.Ln)
            nc.vector.tensor_tensor(out=L2[:], in0=L2[:], in1=pm[:, 0:Gd],
                                    op=mybir.AluOpType.mult)
            t2p = gb.tile([P, 1], mybir.dt.float32)
            nc.vector.tensor_reduce(out=t2p[:], in_=L2[:],
                                    axis=mybir.AxisListType.X,
                                    op=mybir.AluOpType.add)

            # s-space argmax one-hot
            Rv19 = REF[:].rearrange("p (g c) -> p g c", c=E64)[:, :, 0:19]
            m2 = gb.tile([P, Gs], mybir.dt.float32)
            nc.vector.tensor_reduce(out=m2[:], in_=Rv19,
                                    axis=mybir.AxisListType.X,
                                    op=mybir.AluOpType.max)
            eq2 = gb.tile([P, Gs * 19], mybir.dt.float32)
            eq2v = eq2[:].rearrange("p (g c) -> p g c", c=19)
            nc.vector.tensor_tensor(
                out=eq2v, in0=Rv19,
                in1=m2[:, :, None].to_broadcast([P, Gs, 19]),
                op=mybir.AluOpType.is_ge,
            )

            # route other rows into s-slot order; chunked to stay under the
            # 1024-descriptor SWDGE carveout per instruction, with the term1
            # pairing issued per chunk so the tail stays short
            rbase = (Gd + Gs) * 8
            t1g = gb.tile([P, Gs], mybir.dt.float32)
            RCH = 8
            for lo in range(0, Gs, RCH):
                hi = min(lo + RCH, Gs)
                w = hi - lo
                nc.gpsimd.dma_gather(
                    out_ap=R2S[:, lo * E64:hi * E64].rearrange(
                        "p (g c) -> p g c", c=E64),
                    in_ap=scratch[:, :],
                    idxs_ap=ix[:, rbase + lo * 8:rbase + hi * 8],
                    num_idxs=w * P,
                    num_idxs_reg=w * P,
                    elem_size=E64,
                )
                R2v = R2S[:, lo * E64:hi * E64].rearrange(
                    "p (g c) -> p g c", c=E64)[:, :, 0:19]
                eqc = eq2[:, lo * 19:hi * 19].rearrange(
                    "p (g c) -> p g c", c=19)
                # term1 = sum one-hot . other_vec (per chunk)
                nc.vector.tensor_tensor(out=eqc, in0=eqc, in1=R2v,
                                        op=mybir.AluOpType.mult)
                nc.vector.tensor_reduce(out=t1g[:, lo:hi], in_=eqc,
                                        axis=mybir.AxisListType.X,
                                        op=mybir.AluOpType.add)
            nc.vector.tensor_tensor(out=t1g[:], in0=t1g[:], in1=pm[:, Gd:],
                                    op=mybir.AluOpType.mult)
            t1p = gb.tile([P, 1], mybir.dt.float32)
            nc.vector.tensor_reduce(out=t1p[:], in_=t1g[:],
                                    axis=mybir.AxisListType.X,
                                    op=mybir.AluOpType.add)

            pout = psum.tile([1, 2], mybir.dt.float32, space="PSUM")
            nc.tensor.matmul(out=pout[:, 0:1], lhsT=t1p[:], rhs=ones[:],
                             start=True, stop=True)
            nc.tensor.matmul(out=pout[:, 1:2], lhsT=t2p[:], rhs=ones[:],
                             start=True, stop=True)
            so_out = cons.tile([1, 2], mybir.dt.float32)
            nc.vector.tensor_copy(out=so_out[:], in_=pout[:])
            nc.sync.dma_start(out=out[:, :], in_=so_out[:])

    nc.finalize()
    return nc


def _get_program(key):
    if key not in _programs:
        _programs[key] = _build_program(key)
    return _programs[key]


def _host_prep(inds_ref, inds_other):
    """Index-only host math: dedup scatter (last wins), split per core,
    sort both slot spaces by window, build the routing index."""
    ir = np.asarray(inds_ref).astype(np.int64)      # [B, 2, N]
    io = np.asarray(inds_other).astype(np.int64)
    valid = ((ir[:, 0] >= 0) & (ir[:, 0] < W) & (ir[:, 1] >= 0) & (ir[:, 1] < H)
             & (io[:, 0] >= 0) & (io[:, 0] < W) & (io[:, 1] >= 0)
             & (io[:, 1] < H))                       # [B, N]
    lin_ref = H * ir[:, 1] + ir[:, 0]                # [B, N]
    lin_other = H * io[:, 1] + io[:, 0]

    per_core = []
    count = 0
    for b in range(B):
        v = valid[b]
        lo = lin_other[b][v]
        lr = np.clip(lin_ref[b][v], 0, HW - 1)
        u, first_rev = np.unique(lo[::-1], return_index=True)
        d_arr = u.astype(np.int64)
        s_arr = lr[len(lo) - 1 - first_rev].astype(np.int64)
        count += len(u)
        for h in range(2):
            sel = (s_arr // NPIX_H) == h
            s_local = s_arr[sel] - h * NPIX_H
            d_sel = d_arr[sel]
            ks = s_local // W_ROWS
            kd = d_sel // W_ROWS
            s_ord = np.argsort(ks, kind='stable')
            d_ord = np.argsort(kd, kind='stable')
            per_core.append({
                "s": s_local[s_ord], "d": d_sel[d_ord],
                # for each s-sorted position, the d-sorted position of the
                # same correspondence (routing for the regather)
                "route": np.argsort(d_ord, kind='stable')[s_ord],
                "nks": np.bincount(ks, minlength=NW_S).astype(int),
                "nkd": np.bincount(kd, minlength=NW_D).astype(int),
            })
    return per_core, count


def _plan(per_core):
    nks = np.stack([pc["nks"] for pc in per_core])
    nkd = np.stack([pc["nkd"] for pc in per_core])
    GS = np.maximum(1, -(-nks.max(axis=0) // P))
    GD = np.maximum(1, -(-nkd.max(axis=0) // P))
    return (tuple(int(g) for g in GS), tuple(int(g) for g in GD))


def _wrap16(vals, ncols8):
    """Pack an idx stream (concatenated per-window, each padded) into the
    16-partition-wrapped int16 layout [16, ncols8]."""
    outp = np.zeros((16, ncols8), dtype=np.int16)
    j = np.arange(len(vals))
    outp[j % 16, j // 16] = vals.astype(np.int16)
    return outp


def _pack_core(pc, key):
    GS, GD = np.asarray(key[0]), np.asarray(key[1])
    Gs, Gd = int(GS.sum()), int(GD.sum())
    offs = np.concatenate([[0], np.cumsum(GS)]).astype(int)
    offd = np.concatenate([[0], np.cumsum(GD)]).astype(int)

    # slot -> window-local idx streams, padded with 0 per window
    d_stream = np.zeros(Gd * P, dtype=np.int64)
    pm_d = np.zeros((P, Gd), dtype=np.float32)
    # d-sorted position -> d-slot linear index (p*Gd + g) for the routing
    dpos2lin = np.zeros(len(pc["d"]), dtype=np.int64)
    pos = 0
    for m in range(NW_D):
        n = int(pc["nkd"][m])
        jj = np.arange(n)
        slot = offd[m] * P + jj
        d_stream[slot] = pc["d"][pos:pos + n] - m * W_ROWS
        g = offd[m] + jj // P
        pm_d[jj % P, g] = 1.0
        dpos2lin[pos:pos + n] = (jj % P) * Gd + g
        pos += n

    s_stream = np.zeros(Gs * P, dtype=np.int64)
    r_stream = np.zeros(Gs * P, dtype=np.int64)
    pm_s = np.zeros((P, Gs), dtype=np.float32)
    pos = 0
    for k in range(NW_S):
        n = int(pc["nks"][k])
        jj = np.arange(n)
        slot = offs[k] * P + jj
        s_stream[slot] = pc["s"][pos:pos + n] - k * W_ROWS
        r_stream[slot] = dpos2lin[pc["route"][pos:pos + n]]
        pm_s[jj % P, offs[k] + jj // P] = 1.0
        pos += n

    idx16 = np.concatenate([
        _wrap16(d_stream, Gd * 8),
        _wrap16(s_stream, Gs * 8),
        _wrap16(r_stream, Gs * 8),
    ], axis=1)
    idx16 = np.tile(idx16, (8, 1))
    pmio = np.concatenate([pm_d, pm_s], axis=1)
    return idx16, pmio


def _make_in_maps(inputs_ref, inputs_other, per_core, key):
    ref_flat = inputs_ref.reshape(B, C, HW)
    other_flat = inputs_other.reshape(B, C, HW)
    other_cache = {}
    in_maps = []
    for ci, pc in enumerate(per_core):
        b, h = ci // 2, ci % 2
        ref64 = np.empty((NPIX_H, E64), dtype=np.float32)
        ref64[:, :C] = ref_flat[b, :, h * NPIX_H:(h + 1) * NPIX_H].T
        if b not in other_cache:
            o64 = np.empty((NPIX, E64), dtype=np.float32)
            o64[:, :C] = other_flat[b, :, :NPIX].T
            other_cache[b] = o64
        idx16, pmio = _pack_core(pc, key)
        in_maps.append({
            "ref64": ref64,
            "other64": other_cache[b],
            "idx16": idx16,
            "pmio": pmio,
        })
    return in_maps


def kernel(inputs_ref, inputs_other, inds_ref, inds_other, weights):
    from concourse.bass_utils import run_bass_kernel_spmd

    inputs_ref = np.asarray(inputs_ref, dtype=np.float32)
    inputs_other = np.asarray(inputs_other, dtype=np.float32)

    per_core, count = _host_prep(inds_ref, inds_other)
    key = _plan(per_core)
    nc = _get_program(key)

    in_maps = _make_in_maps(inputs_ref, inputs_other, per_core, key)
    res = run_bass_kernel_spmd(nc, in_maps, core_ids=list(range(NCORES)))
    total = 0.0
    for r in res.results:
        o = np.asarray(r["out"], dtype=np.float64)
        total += o[0, 0] - o[0, 1]
    loss = -total / max(count, 1)
    return np.float32(loss)


# revision 8
# speedup vs baseline: 2.9289x; 1.0526x over previous
"""Trainium2 Bass kernel for nn_CorrClassLoss.

Reference computation (B=4, C=19, H=512, W=1024, N=5000, IGNORE=255):
  ref_class = argmax_c inputs_ref[b].reshape(C, H*W)      # flat W-major
  lin_ref   = 512*y_ref + x_ref    (NOTE: linearized with H, kept faithfully)
  lin_other = 512*y_other + x_other
  gathered  = ref_class[b, lin_ref]
  target[b, lin_other] = gathered  (scatter, last write wins; rest IGNORE)
  loss = mean over non-ignored pixels of -log_softmax(inputs_other)[b, target, px]

Only flat positions [0, 262144) are touched; at most N unique scatter dests
per batch contribute:

  loss = -(1/cnt) * sum over unique dests d (last writer j, src s_j) of
         [ x_other[b, cls(s_j), d] - ln(sum_c exp(x_other[b, c, d])) ]
  cls(s) = argmax_c x_ref[b, c, s],  cnt = total unique dests.

Strategy (8 cores, data-parallel over (batch, half-of-sources)). Host does
index-only math (dedup last-wins, core split, window sort, idx packing) plus
pure relayout of image data (pixel-major transpose into 64-slot rows).
Device per core, all value compute on device:
  - 8x InstDMAGatherAnt fetch other rows (64-f32 slots, int16 window-local
    idx) into a d-sorted slot space [128, Gd] (slot j = [j%128, j//128]);
    logsumexp term ln(sum_c exp(.)) reduced there (masked by pm_d).
  - the fetched other rows are dumped to a DRAM scratch (one strided DMA)
    and regathered (1x InstDMAGatherAnt) into the s-sorted slot space
    [128, Gs] built by 4x InstDMAGatherAnt of ref rows, where the argmax
    one-hot (max + is_ge on strided 19-of-64 APs) pairs with them:
    term1 = sum onehot . other_vec (masked by pm_s).
  Output [1, 2] = (term1_sum, term2_sum);
  host: loss = -(sum_cores term1 - term2) / cnt.
"""

import sys

if "/opt/trn_rl_repo" not in sys.path:
    sys.path.insert(0, "/opt/trn_rl_repo")

import numpy as np

B, C, H, W = 4, 19, 512, 1024
HW = H * W                 # 524288
NPIX = 262144              # touched flat range [0, 262144)
NPIX_H = NPIX // 2         # 131072 source pixels per core
N = 5000
NCORES = 8

P = 128                    # partitions
E64 = 64                   # f32 slots per table row (256B, dma_gather unit)
W_ROWS = 32768             # rows per dma_gather window (int16 idx range)
NW_S = NPIX_H // W_ROWS    # 4 source windows per core
NW_D = NPIX // W_ROWS      # 8 dest windows per core

_programs = {}


def _build_program(key):
    import concourse.bass as bass
    import concourse.bacc as bacc
    import concourse.mybir as mybir
    import concourse.tile as tile

    GS = list(key[0])          # columns per source window
    GD = list(key[1])          # columns per dest window
    Gs, Gd = sum(GS), sum(GD)
    offs = np.concatenate([[0], np.cumsum(GS)]).astype(int)
    offd = np.concatenate([[0], np.cumsum(GD)]).astype(int)

    nc = bacc.Bacc("TRN2", target_bir_lowering=False, debug=False,
                   num_devices=NCORES)

    ref64 = nc.dram_tensor("ref64", [NPIX_H, E64], mybir.dt.float32,
                           kind="ExternalInput")
    other64 = nc.dram_tensor("other64", [NPIX, E64], mybir.dt.float32,
                             kind="ExternalInput")
    # idx streams (int16, 16-wrapped, replicated x8): [d | s | r]
    idx16 = nc.dram_tensor("idx16", [P, (Gd + 2 * Gs) * 8], mybir.dt.int16,
                           kind="ExternalInput")
    # valid masks: [pm_d | pm_s]
    pmio = nc.dram_tensor("pmio", [P, Gd + Gs], mybir.dt.float32,
                          kind="ExternalInput")
    scratch = nc.dram_tensor("scratch", [P * Gd, E64], mybir.dt.float32,
                             kind="Internal")
    out = nc.dram_tensor("out", [P, 2], mybir.dt.float32,
                         kind="ExternalOutput")

    with tile.TileContext(nc) as tc:
        with (
            tc.tile_pool(name="gb", bufs=1) as gb,
            tc.tile_pool(name="cons", bufs=1) as cons,
            tc.tile_pool(name="psum", bufs=1, space="PSUM") as psum,
        ):
            ix = gb.tile([P, (Gd + 2 * Gs) * 8], mybir.dt.int16)
            nc.sync.dma_start(out=ix[:], in_=idx16[:, :])
            pm = gb.tile([P, Gd + Gs], mybir.dt.float32)
            nc.sync.dma_start(out=pm[:], in_=pmio[:, :])

            OTH = gb.tile([P, Gd * E64], mybir.dt.float32)
            REF = gb.tile([P, Gs * E64], mybir.dt.float32)
            R2S = gb.tile([P, Gs * E64], mybir.dt.float32)

            # d-side: one gather per 32K-row dest window (critical chain:
            # feeds the scratch dump + regather)
            for m in range(NW_D):
                if GD[m] == 0:
                    continue
                nc.gpsimd.dma_gather(
                    out_ap=OTH[:, offd[m] * E64:offd[m + 1] * E64].rearrange(
                        "p (g c) -> p g c", c=E64),
                    in_ap=other64[m * W_ROWS:(m + 1) * W_ROWS, :],
                    idxs_ap=ix[:, offd[m] * 8:offd[m + 1] * 8],
                    num_idxs=GD[m] * P,
                    num_idxs_reg=GD[m] * P,
                    elem_size=E64,
                )
            # s-side: ref rows into the s-sorted slot space (fills the Pool
            # gap while the dump completes)
            sbase = Gd * 8
            for k in range(NW_S):
                if GS[k] == 0:
                    continue
                nc.gpsimd.dma_gather(
                    out_ap=REF[:, offs[k] * E64:offs[k + 1] * E64].rearrange(
                        "p (g c) -> p g c", c=E64),
                    in_ap=ref64[k * W_ROWS:(k + 1) * W_ROWS, :],
                    idxs_ap=ix[:, sbase + offs[k] * 8:sbase + offs[k + 1] * 8],
                    num_idxs=GS[k] * P,
                    num_idxs_reg=GS[k] * P,
                    elem_size=E64,
                )

            OTHv = OTH[:].rearrange("p (g c) -> p g c", c=E64)[:, :, 0:19]
            # dump the useful 19-of-64 of each fetched other row to scratch
            # row (p*Gd + g); regathers below route them to s-slot order.
            # One dump per dest window so each issues as soon as its gather
            # lands instead of waiting for all eight.
            scr3 = scratch.rearrange("(p g) c -> p g c", g=Gd)
            for m in range(NW_D):
                if GD[m] == 0:
                    continue
                nc.sync.dma_start(
                    out=scr3[:, offd[m]:offd[m + 1], 0:19],
                    in_=OTHv[:, offd[m]:offd[m + 1], :],
                )

            # term2 in d-space: ln(sum_c exp(other_vec)), masked
            e2 = gb.tile([P, Gd * 19], mybir.dt.float32)
            e2v = e2[:].rearrange("p (g c) -> p g c", c=19)
            nc.scalar.activation(e2v, OTHv, mybir.ActivationFunctionType.Exp)
            S2 = gb.tile([P, Gd], mybir.dt.float32)
            nc.vector.tensor_reduce(out=S2[:], in_=e2v,
                                    axis=mybir.AxisListType.X,
                                    op=mybir.AluOpType.add)
            L2 = gb.tile([P, Gd], mybir.dt.float32)
            nc.scalar.activation(L2[:], S2[:], mybir.ActivationFunctionType.Ln)
            nc.vector.tensor_tensor(out=L2[:], in0=L2[:], in1=pm[:, 0:Gd],
                                    op=mybir.AluOpType.mult)
            tp = cons.tile([P, 2], mybir.dt.float32)
            nc.vector.tensor_reduce(out=tp[:, 1:2], in_=L2[:],
                                    axis=mybir.AxisListType.X,
                                    op=mybir.AluOpType.add)

            # s-space argmax one-hot
            Rv19 = REF[:].rearrange("p (g c) -> p g c", c=E64)[:, :, 0:19]
            m2 = gb.tile([P, Gs], mybir.dt.float32)
            nc.vector.tensor_reduce(out=m2[:], in_=Rv19,
                                    axis=mybir.AxisListType.X,
                                    op=mybir.AluOpType.max)
            eq2 = gb.tile([P, Gs * 19], mybir.dt.float32)
            eq2v = eq2[:].rearrange("p (g c) -> p g c", c=19)
            nc.vector.tensor_tensor(
                out=eq2v, in0=Rv19,
                in1=m2[:, :, None].to_broadcast([P, Gs, 19]),
                op=mybir.AluOpType.is_ge,
            )
            nc.vector.tensor_tensor(
                out=eq2v, in0=eq2v,
                in1=pm[:, Gd:, None].to_broadcast([P, Gs, 19]),
                op=mybir.AluOpType.mult,
            )

            # route other rows into s-slot order; chunked to stay under the
            # 1024-descriptor SWDGE carveout per instruction, with the term1
            # pairing issued per chunk so the tail stays short
            rbase = (Gd + Gs) * 8
            t1g = gb.tile([P, Gs], mybir.dt.float32)
            RCH = 8
            for lo in range(0, Gs, RCH):
                hi = min(lo + RCH, Gs)
                w = hi - lo
                nc.gpsimd.dma_gather(
                    out_ap=R2S[:, lo * E64:hi * E64].rearrange(
                        "p (g c) -> p g c", c=E64),
                    in_ap=scratch[:, :],
                    idxs_ap=ix[:, rbase + lo * 8:rbase + hi * 8],
                    num_idxs=w * P,
                    num_idxs_reg=w * P,
                    elem_size=E64,
                )
                R2v = R2S[:, lo * E64:hi * E64].rearrange(
                    "p (g c) -> p g c", c=E64)[:, :, 0:19]
                eqc = eq2[:, lo * 19:hi * 19].rearrange(
                    "p (g c) -> p g c", c=19)
                # term1 = sum one-hot . other_vec (per chunk)
                nc.vector.tensor_tensor(out=eqc, in0=eqc, in1=R2v,
                                        op=mybir.AluOpType.mult)
                nc.vector.tensor_reduce(out=t1g[:, lo:hi], in_=eqc,
                                        axis=mybir.AxisListType.X,
                                        op=mybir.AluOpType.add)
            nc.vector.tensor_reduce(out=tp[:, 0:1], in_=t1g[:],
                                    axis=mybir.AxisListType.X,
                                    op=mybir.AluOpType.add)
            nc.sync.dma_start(out=out[:, :], in_=tp[:])

    nc.finalize()
    return nc


def _get_program(key):
    if key not in _programs:
        _programs[key] = _build_program(key)
    return _programs[key]


def _host_prep(inds_ref, inds_other):
    """Index-only host math: dedup scatter (last wins), split per core,
    sort both slot spaces by window, build the routing index."""
    ir = np.asarray(inds_ref).astype(np.int64)      # [B, 2, N]
    io = np.asarray(inds_other).astype(np.int64)
    valid = ((ir[:, 0] >= 0) & (ir[:, 0] < W) & (ir[:, 1] >= 0) & (ir[:, 1] < H)
             & (io[:, 0] >= 0) & (io[:, 0] < W) & (io[:, 1] >= 0)
             & (io[:, 1] < H))                       # [B, N]
    lin_ref = H * ir[:, 1] + ir[:, 0]                # [B, N]
    lin_other = H * io[:, 1] + io[:, 0]

    per_core = []
    count = 0
    for b in range(B):
        v = valid[b]
        lo = lin_other[b][v]
        lr = np.clip(lin_ref[b][v], 0, HW - 1)
        u, first_rev = np.unique(lo[::-1], return_index=True)
        d_arr = u.astype(np.int64)
        s_arr = lr[len(lo) - 1 - first_rev].astype(np.int64)
        count += len(u)
        for h in range(2):
            sel = (s_arr // NPIX_H) == h
            s_local = s_arr[sel] - h * NPIX_H
            d_sel = d_arr[sel]
            ks = s_local // W_ROWS
            kd = d_sel // W_ROWS
            s_ord = np.argsort(ks, kind='stable')
            d_ord = np.argsort(kd, kind='stable')
            per_core.append({
                "s": s_local[s_ord], "d": d_sel[d_ord],
                # for each s-sorted position, the d-sorted position of the
                # same correspondence (routing for the regather)
                "route": np.argsort(d_ord, kind='stable')[s_ord],
                "nks": np.bincount(ks, minlength=NW_S).astype(int),
                "nkd": np.bincount(kd, minlength=NW_D).astype(int),
            })
    return per_core, count


def _plan(per_core):
    nks = np.stack([pc["nks"] for pc in per_core])
    nkd = np.stack([pc["nkd"] for pc in per_core])
    GS = np.maximum(1, -(-nks.max(axis=0) // P))
    GD = np.maximum(1, -(-nkd.max(axis=0) // P))
    return (tuple(int(g) for g in GS), tuple(int(g) for g in GD))


def _wrap16(vals, ncols8):
    """Pack an idx stream (concatenated per-window, each padded) into the
    16-partition-wrapped int16 layout [16, ncols8]."""
    outp = np.zeros((16, ncols8), dtype=np.int16)
    j = np.arange(len(vals))
    outp[j % 16, j // 16] = vals.astype(np.int16)
    return outp


def _pack_core(pc, key):
    GS, GD = np.asarray(key[0]), np.asarray(key[1])
    Gs, Gd = int(GS.sum()), int(GD.sum())
    offs = np.concatenate([[0], np.cumsum(GS)]).astype(int)
    offd = np.concatenate([[0], np.cumsum(GD)]).astype(int)

    # slot -> window-local idx streams, padded with 0 per window
    d_stream = np.zeros(Gd * P, dtype=np.int64)
    pm_d = np.zeros((P, Gd), dtype=np.float32)
    # d-sorted position -> d-slot linear index (p*Gd + g) for the routing
    dpos2lin = np.zeros(len(pc["d"]), dtype=np.int64)
    pos = 0
    for m in range(NW_D):
        n = int(pc["nkd"][m])
        jj = np.arange(n)
        slot = offd[m] * P + jj
        d_stream[slot] = pc["d"][pos:pos + n] - m * W_ROWS
        g = offd[m] + jj // P
        pm_d[jj % P, g] = 1.0
        dpos2lin[pos:pos + n] = (jj % P) * Gd + g
        pos += n

    s_stream = np.zeros(Gs * P, dtype=np.int64)
    r_stream = np.zeros(Gs * P, dtype=np.int64)
    pm_s = np.zeros((P, Gs), dtype=np.float32)
    pos = 0
    for k in range(NW_S):
        n = int(pc["nks"][k])
        jj = np.arange(n)
        slot = offs[k] * P + jj
        s_stream[slot] = pc["s"][pos:pos + n] - k * W_ROWS
        r_stream[slot] = dpos2lin[pc["route"][pos:pos + n]]
        pm_s[jj % P, offs[k] + jj // P] = 1.0
        pos += n

    idx16 = np.concatenate([
        _wrap16(d_stream, Gd * 8),
        _wrap16(s_stream, Gs * 8),
        _wrap16(r_stream, Gs * 8),
    ], axis=1)
    idx16 = np.tile(idx16, (8, 1))
    pmio = np.concatenate([pm_d, pm_s], axis=1)
    return idx16, pmio


def _make_in_maps(inputs_ref, inputs_other, per_core, key):
    ref_flat = inputs_ref.reshape(B, C, HW)
    other_flat = inputs_other.reshape(B, C, HW)
    other_cache = {}
    in_maps = []
    for ci, pc in enumerate(per_core):
        b, h = ci // 2, ci % 2
        ref64 = np.empty((NPIX_H, E64), dtype=np.float32)
        ref64[:, :C] = ref_flat[b, :, h * NPIX_H:(h + 1) * NPIX_H].T
        if b not in other_cache:
            o64 = np.empty((NPIX, E64), dtype=np.float32)
            o64[:, :C] = other_flat[b, :, :NPIX].T
            other_cache[b] = o64
        idx16, pmio = _pack_core(pc, key)
        in_maps.append({
            "ref64": ref64,
            "other64": other_cache[b],
            "idx16": idx16,
            "pmio": pmio,
        })
    return in_maps


def kernel(inputs_ref, inputs_other, inds_ref, inds_other, weights):
    from concourse.bass_utils import run_bass_kernel_spmd

    inputs_ref = np.asarray(inputs_ref, dtype=np.float32)
    inputs_other = np.asarray(inputs_other, dtype=np.float32)

    per_core, count = _host_prep(inds_ref, inds_other)
    key = _plan(per_core)
    nc = _get_program(key)

    in_maps = _make_in_maps(inputs_ref, inputs_other, per_core, key)
    res = run_bass_kernel_spmd(nc, in_maps, core_ids=list(range(NCORES)))
    total = 0.0
    for r in res.results:
        o = np.asarray(r["out"], dtype=np.float64)
        total += o[:, 0].sum() - o[:, 1].sum()
    loss = -total / max(count, 1)
    return np.float32(loss)


# revision 9
# speedup vs baseline: 2.9322x; 1.0011x over previous
"""Trainium2 Bass kernel for nn_CorrClassLoss.

Reference computation (B=4, C=19, H=512, W=1024, N=5000, IGNORE=255):
  ref_class = argmax_c inputs_ref[b].reshape(C, H*W)      # flat W-major
  lin_ref   = 512*y_ref + x_ref    (NOTE: linearized with H, kept faithfully)
  lin_other = 512*y_other + x_other
  gathered  = ref_class[b, lin_ref]
  target[b, lin_other] = gathered  (scatter, last write wins; rest IGNORE)
  loss = mean over non-ignored pixels of -log_softmax(inputs_other)[b, target, px]

Only flat positions [0, 262144) are touched; at most N unique scatter dests
per batch contribute:

  loss = -(1/cnt) * sum over unique dests d (last writer j, src s_j) of
         [ x_other[b, cls(s_j), d] - ln(sum_c exp(x_other[b, c, d])) ]
  cls(s) = argmax_c x_ref[b, c, s],  cnt = total unique dests.

Strategy (8 cores, data-parallel over (batch, half-of-sources)). Host does
index-only math (dedup last-wins, core split, window sort, idx packing) plus
pure relayout of image data (pixel-major transpose into 64-slot rows).
Device per core, all value compute on device:
  - 8x InstDMAGatherAnt fetch other rows (64-f32 slots, int16 window-local
    idx) into a d-sorted slot space [128, Gd] (slot j = [j%128, j//128]);
    logsumexp term ln(sum_c exp(.)) reduced there (masked by pm_d).
  - the fetched other rows are dumped to a DRAM scratch (one strided DMA)
    and regathered (1x InstDMAGatherAnt) into the s-sorted slot space
    [128, Gs] built by 4x InstDMAGatherAnt of ref rows, where the argmax
    one-hot (max + is_ge on strided 19-of-64 APs) pairs with them:
    term1 = sum onehot . other_vec (masked by pm_s).
  Output [1, 2] = (term1_sum, term2_sum);
  host: loss = -(sum_cores term1 - term2) / cnt.
"""

import sys

if "/opt/trn_rl_repo" not in sys.path:
    sys.path.insert(0, "/opt/trn_rl_repo")

import numpy as np

B, C, H, W = 4, 19, 512, 1024
HW = H * W                 # 524288
NPIX = 262144              # touched flat range [0, 262144)
NPIX_H = NPIX // 2         # 131072 source pixels per core
N = 5000
NCORES = 8

P = 128                    # partitions
E64 = 64                   # f32 slots per table row (256B, dma_gather unit)
W_ROWS = 32768             # rows per dma_gather window (int16 idx range)
NW_S = NPIX_H // W_ROWS    # 4 source windows per core
NW_D = NPIX // W_ROWS      # 8 dest windows per core

_programs = {}


def _build_program(key):
    import concourse.bass as bass
    import concourse.bacc as bacc
    import concourse.mybir as mybir
    import concourse.tile as tile

    GS = list(key[0])          # columns per source window
    GD = list(key[1])          # columns per dest window
    Gs, Gd = sum(GS), sum(GD)
    offs = np.concatenate([[0], np.cumsum(GS)]).astype(int)
    offd = np.concatenate([[0], np.cumsum(GD)]).astype(int)

    nc = bacc.Bacc("TRN2", target_bir_lowering=False, debug=False,
                   num_devices=NCORES)

    ref64 = nc.dram_tensor("ref64", [NPIX_H, E64], mybir.dt.float32,
                           kind="ExternalInput")
    other64 = nc.dram_tensor("other64", [NPIX, E64], mybir.dt.float32,
                             kind="ExternalInput")
    # idx streams (int16, 16-wrapped, replicated x8): [d | s | r]
    idx16 = nc.dram_tensor("idx16", [P, (Gd + 2 * Gs) * 8], mybir.dt.int16,
                           kind="ExternalInput")
    # valid masks: [pm_d | pm_s]
    pmio = nc.dram_tensor("pmio", [P, Gd + Gs], mybir.dt.float32,
                          kind="ExternalInput")
    scratch = nc.dram_tensor("scratch", [P * Gd, E64], mybir.dt.float32,
                             kind="Internal")
    out = nc.dram_tensor("out", [P, Gs + Gd], mybir.dt.float32,
                         kind="ExternalOutput")

    with tile.TileContext(nc) as tc:
        with (
            tc.tile_pool(name="gb", bufs=1) as gb,
            tc.tile_pool(name="cons", bufs=1) as cons,
            tc.tile_pool(name="psum", bufs=1, space="PSUM") as psum,
        ):
            ix = gb.tile([P, (Gd + 2 * Gs) * 8], mybir.dt.int16)
            nc.sync.dma_start(out=ix[:], in_=idx16[:, :])
            pm = gb.tile([P, Gd + Gs], mybir.dt.float32)
            nc.sync.dma_start(out=pm[:], in_=pmio[:, :])

            OTH = gb.tile([P, Gd * E64], mybir.dt.float32)
            REF = gb.tile([P, Gs * E64], mybir.dt.float32)
            R2S = gb.tile([P, Gs * E64], mybir.dt.float32)

            # d-side: one gather per 32K-row dest window (critical chain:
            # feeds the scratch dump + regather)
            for m in range(NW_D):
                if GD[m] == 0:
                    continue
                nc.gpsimd.dma_gather(
                    out_ap=OTH[:, offd[m] * E64:offd[m + 1] * E64].rearrange(
                        "p (g c) -> p g c", c=E64),
                    in_ap=other64[m * W_ROWS:(m + 1) * W_ROWS, :],
                    idxs_ap=ix[:, offd[m] * 8:offd[m + 1] * 8],
                    num_idxs=GD[m] * P,
                    num_idxs_reg=GD[m] * P,
                    elem_size=E64,
                )
            OTHv = OTH[:].rearrange("p (g c) -> p g c", c=E64)[:, :, 0:19]
            # dump the useful 19-of-64 of each fetched other row to scratch
            # row (p*Gd + g); regathers below route them to s-slot order.
            # One dump per dest window so each issues as soon as its gather
            # lands instead of waiting for all eight.
            scr3 = scratch.rearrange("(p g) c -> p g c", g=Gd)
            for m0 in range(0, NW_D, 4):
                m1 = min(m0 + 4, NW_D)
                if offd[m1] == offd[m0]:
                    continue
                nc.sync.dma_start(
                    out=scr3[:, offd[m0]:offd[m1], 0:19],
                    in_=OTHv[:, offd[m0]:offd[m1], :],
                )

            # s-side: ref rows into the s-sorted slot space (fills the Pool
            # gap while the dump completes)
            sbase = Gd * 8
            for k in range(NW_S):
                if GS[k] == 0:
                    continue
                nc.gpsimd.dma_gather(
                    out_ap=REF[:, offs[k] * E64:offs[k + 1] * E64].rearrange(
                        "p (g c) -> p g c", c=E64),
                    in_ap=ref64[k * W_ROWS:(k + 1) * W_ROWS, :],
                    idxs_ap=ix[:, sbase + offs[k] * 8:sbase + offs[k + 1] * 8],
                    num_idxs=GS[k] * P,
                    num_idxs_reg=GS[k] * P,
                    elem_size=E64,
                )

            # term2 in d-space: ln(sum_c exp(other_vec)), masked
            e2 = gb.tile([P, Gd * 19], mybir.dt.float32)
            e2v = e2[:].rearrange("p (g c) -> p g c", c=19)
            nc.scalar.activation(e2v, OTHv, mybir.ActivationFunctionType.Exp)
            S2 = gb.tile([P, Gd], mybir.dt.float32)
            nc.vector.tensor_reduce(out=S2[:], in_=e2v,
                                    axis=mybir.AxisListType.X,
                                    op=mybir.AluOpType.add)
            TG = cons.tile([P, Gs + Gd], mybir.dt.float32)
            L2 = TG[:, Gs:]
            nc.scalar.activation(L2, S2[:], mybir.ActivationFunctionType.Ln)
            nc.vector.tensor_tensor(out=L2, in0=L2, in1=pm[:, 0:Gd],
                                    op=mybir.AluOpType.mult)

            # s-space argmax one-hot, issued per source window so each
            # starts as soon as its gather lands (pm_s folded in here)
            m2 = gb.tile([P, Gs], mybir.dt.float32)
            eq2 = gb.tile([P, Gs * 19], mybir.dt.float32)
            for k in range(NW_S):
                lo, hi = int(offs[k]), int(offs[k + 1])
                if hi == lo:
                    continue
                Rw = REF[:, lo * E64:hi * E64].rearrange(
                    "p (g c) -> p g c", c=E64)[:, :, 0:19]
                nc.vector.tensor_reduce(out=m2[:, lo:hi], in_=Rw,
                                        axis=mybir.AxisListType.X,
                                        op=mybir.AluOpType.max)
                ew = eq2[:, lo * 19:hi * 19].rearrange(
                    "p (g c) -> p g c", c=19)
                nc.vector.tensor_tensor(
                    out=ew, in0=Rw,
                    in1=m2[:, lo:hi, None].to_broadcast([P, hi - lo, 19]),
                    op=mybir.AluOpType.is_ge,
                )
                nc.vector.tensor_tensor(
                    out=ew, in0=ew,
                    in1=pm[:, Gd + lo:Gd + hi, None].to_broadcast(
                        [P, hi - lo, 19]),
                    op=mybir.AluOpType.mult,
                )

            # route other rows into s-slot order; chunked to stay under the
            # 1024-descriptor SWDGE carveout per instruction, with the term1
            # pairing issued per chunk so the tail stays short
            rbase = (Gd + Gs) * 8
            t1g = TG[:, 0:Gs]
            RCH = 8
            for lo in range(0, Gs, RCH):
                hi = min(lo + RCH, Gs)
                w = hi - lo
                nc.gpsimd.dma_gather(
                    out_ap=R2S[:, lo * E64:hi * E64].rearrange(
                        "p (g c) -> p g c", c=E64),
                    in_ap=scratch[:, :],
                    idxs_ap=ix[:, rbase + lo * 8:rbase + hi * 8],
                    num_idxs=w * P,
                    num_idxs_reg=w * P,
                    elem_size=E64,
                )
                R2v = R2S[:, lo * E64:hi * E64].rearrange(
                    "p (g c) -> p g c", c=E64)[:, :, 0:19]
                eqc = eq2[:, lo * 19:hi * 19].rearrange(
                    "p (g c) -> p g c", c=19)
                # term1 = sum one-hot . other_vec (per chunk)
                nc.vector.tensor_tensor(out=eqc, in0=eqc, in1=R2v,
                                        op=mybir.AluOpType.mult)
                nc.vector.tensor_reduce(out=t1g[:, lo:hi], in_=eqc,
                                        axis=mybir.AxisListType.X,
                                        op=mybir.AluOpType.add)
            nc.sync.dma_start(out=out[:, :], in_=TG[:])

    nc.finalize()
    return nc


def _get_program(key):
    if key not in _programs:
        _programs[key] = _build_program(key)
    return _programs[key]


def _host_prep(inds_ref, inds_other):
    """Index-only host math: dedup scatter (last wins), split per core,
    sort both slot spaces by window, build the routing index."""
    ir = np.asarray(inds_ref).astype(np.int64)      # [B, 2, N]
    io = np.asarray(inds_other).astype(np.int64)
    valid = ((ir[:, 0] >= 0) & (ir[:, 0] < W) & (ir[:, 1] >= 0) & (ir[:, 1] < H)
             & (io[:, 0] >= 0) & (io[:, 0] < W) & (io[:, 1] >= 0)
             & (io[:, 1] < H))                       # [B, N]
    lin_ref = H * ir[:, 1] + ir[:, 0]                # [B, N]
    lin_other = H * io[:, 1] + io[:, 0]

    per_core = []
    count = 0
    for b in range(B):
        v = valid[b]
        lo = lin_other[b][v]
        lr = np.clip(lin_ref[b][v], 0, HW - 1)
        u, first_rev = np.unique(lo[::-1], return_index=True)
        d_arr = u.astype(np.int64)
        s_arr = lr[len(lo) - 1 - first_rev].astype(np.int64)
        count += len(u)
        for h in range(2):
            sel = (s_arr // NPIX_H) == h
            s_local = s_arr[sel] - h * NPIX_H
            d_sel = d_arr[sel]
            ks = s_local // W_ROWS
            kd = d_sel // W_ROWS
            s_ord = np.argsort(ks, kind='stable')
            d_ord = np.argsort(kd, kind='stable')
            per_core.append({
                "s": s_local[s_ord], "d": d_sel[d_ord],
                # for each s-sorted position, the d-sorted position of the
                # same correspondence (routing for the regather)
                "route": np.argsort(d_ord, kind='stable')[s_ord],
                "nks": np.bincount(ks, minlength=NW_S).astype(int),
                "nkd": np.bincount(kd, minlength=NW_D).astype(int),
            })
    return per_core, count


def _plan(per_core):
    nks = np.stack([pc["nks"] for pc in per_core])
    nkd = np.stack([pc["nkd"] for pc in per_core])
    GS = np.maximum(1, -(-nks.max(axis=0) // P))
    GD = np.maximum(1, -(-nkd.max(axis=0) // P))
    return (tuple(int(g) for g in GS), tuple(int(g) for g in GD))


def _wrap16(vals, ncols8):
    """Pack an idx stream (concatenated per-window, each padded) into the
    16-partition-wrapped int16 layout [16, ncols8]."""
    outp = np.zeros((16, ncols8), dtype=np.int16)
    j = np.arange(len(vals))
    outp[j % 16, j // 16] = vals.astype(np.int16)
    return outp


def _pack_core(pc, key):
    GS, GD = np.asarray(key[0]), np.asarray(key[1])
    Gs, Gd = int(GS.sum()), int(GD.sum())
    offs = np.concatenate([[0], np.cumsum(GS)]).astype(int)
    offd = np.concatenate([[0], np.cumsum(GD)]).astype(int)

    # slot -> window-local idx streams, padded with 0 per window
    d_stream = np.zeros(Gd * P, dtype=np.int64)
    pm_d = np.zeros((P, Gd), dtype=np.float32)
    # d-sorted position -> d-slot linear index (p*Gd + g) for the routing
    dpos2lin = np.zeros(len(pc["d"]), dtype=np.int64)
    pos = 0
    for m in range(NW_D):
        n = int(pc["nkd"][m])
        jj = np.arange(n)
        slot = offd[m] * P + jj
        d_stream[slot] = pc["d"][pos:pos + n] - m * W_ROWS
        g = offd[m] + jj // P
        pm_d[jj % P, g] = 1.0
        dpos2lin[pos:pos + n] = (jj % P) * Gd + g
        pos += n

    s_stream = np.zeros(Gs * P, dtype=np.int64)
    r_stream = np.zeros(Gs * P, dtype=np.int64)
    pm_s = np.zeros((P, Gs), dtype=np.float32)
    pos = 0
    for k in range(NW_S):
        n = int(pc["nks"][k])
        jj = np.arange(n)
        slot = offs[k] * P + jj
        s_stream[slot] = pc["s"][pos:pos + n] - k * W_ROWS
        r_stream[slot] = dpos2lin[pc["route"][pos:pos + n]]
        pm_s[jj % P, offs[k] + jj // P] = 1.0
        pos += n

    idx16 = np.concatenate([
        _wrap16(d_stream, Gd * 8),
        _wrap16(s_stream, Gs * 8),
        _wrap16(r_stream, Gs * 8),
    ], axis=1)
    idx16 = np.tile(idx16, (8, 1))
    pmio = np.concatenate([pm_d, pm_s], axis=1)
    return idx16, pmio


def _make_in_maps(inputs_ref, inputs_other, per_core, key):
    ref_flat = inputs_ref.reshape(B, C, HW)
    other_flat = inputs_other.reshape(B, C, HW)
    other_cache = {}
    in_maps = []
    for ci, pc in enumerate(per_core):
        b, h = ci // 2, ci % 2
        ref64 = np.empty((NPIX_H, E64), dtype=np.float32)
        ref64[:, :C] = ref_flat[b, :, h * NPIX_H:(h + 1) * NPIX_H].T
        if b not in other_cache:
            o64 = np.empty((NPIX, E64), dtype=np.float32)
            o64[:, :C] = other_flat[b, :, :NPIX].T
            other_cache[b] = o64
        idx16, pmio = _pack_core(pc, key)
        in_maps.append({
            "ref64": ref64,
            "other64": other_cache[b],
            "idx16": idx16,
            "pmio": pmio,
        })
    return in_maps


def kernel(inputs_ref, inputs_other, inds_ref, inds_other, weights):
    from concourse.bass_utils import run_bass_kernel_spmd

    inputs_ref = np.asarray(inputs_ref, dtype=np.float32)
    inputs_other = np.asarray(inputs_other, dtype=np.float32)

    per_core, count = _host_prep(inds_ref, inds_other)
    key = _plan(per_core)
    nc = _get_program(key)

    in_maps = _make_in_maps(inputs_ref, inputs_other, per_core, key)
    res = run_bass_kernel_spmd(nc, in_maps, core_ids=list(range(NCORES)))
    total = 0.0
    for r in res.results:
        o = np.asarray(r["out"], dtype=np.float64)
        Gs = sum(key[0])
        total += o[:, :Gs].sum() - o[:, Gs:].sum()
    loss = -total / max(count, 1)
    return np.float32(loss)


# revision 10
# speedup vs baseline: 3.3477x; 1.1417x over previous
"""Trainium2 Bass kernel for nn_CorrClassLoss.

Reference computation (B=4, C=19, H=512, W=1024, N=5000, IGNORE=255):
  ref_class = argmax_c inputs_ref[b].reshape(C, H*W)      # flat W-major
  lin_ref   = 512*y_ref + x_ref    (NOTE: linearized with H, kept faithfully)
  lin_other = 512*y_other + x_other
  gathered  = ref_class[b, lin_ref]
  target[b, lin_other] = gathered  (scatter, last write wins; rest IGNORE)
  loss = mean over non-ignored pixels of -log_softmax(inputs_other)[b, target, px]

Only flat positions [0, 262144) are touched; at most N unique scatter dests
per batch contribute:

  loss = -(1/cnt) * sum over unique dests d (last writer j, src s_j) of
         [ x_other[b, cls(s_j), d] - ln(sum_c exp(x_other[b, c, d])) ]
  cls(s) = argmax_c x_ref[b, c, s],  cnt = total unique dests.

Strategy (8 cores, data-parallel over (batch, half-of-sources)). Host does
index-only math (dedup last-wins, core split, window/parity sort, idx
packing) plus pure relayout of image data (pixel-major transpose into
64-slot rows; two pixels share one 512B table row so one int16-indexed
32K-row gather window covers 64K pixels). Device per core:
  - 4 (dest) + 4 (source) InstDMAGatherAnt fetch two-pixel rows into
    window/parity-sorted slot spaces (slot j = [j%128, j//128]); per-parity
    strided views compact the valid 19 channels into packed tiles.
  - dest side: ln(sum_c exp(.)) (masked) -> term2 partials; the packed
    dest rows are dumped to a DRAM scratch (one strided DMA) and regathered
    (3x InstDMAGatherAnt, chunked under the 1024-descriptor carveout) into
    source-slot order, where the argmax one-hot (max + is_ge) pairs them:
    term1 partials = sum one-hot . other_vec (mask folded into the one-hot).
  Output [128, Gs+Gd] = (term1 partials | term2 partials); host sums and
  computes loss = -(sum t1 - sum t2) / cnt.
"""

import sys

if "/opt/trn_rl_repo" not in sys.path:
    sys.path.insert(0, "/opt/trn_rl_repo")

import numpy as np

B, C, H, W = 4, 19, 512, 1024
HW = H * W                 # 524288
NPIX = 262144              # touched flat range [0, 262144)
NPIX_H = NPIX // 2         # 131072 source pixels per core
N = 5000
NCORES = 8

P = 128                    # partitions
E128 = 128                 # f32 slots per two-pixel table row (512B)
W_ROWS = 32768             # rows per dma_gather window (int16 idx range)
NW_S = NPIX_H // 2 // W_ROWS   # 2 source windows per core
NW_D = NPIX // 2 // W_ROWS     # 4 dest windows per core
NG_S = NW_S * 2            # source (window, parity) groups
NG_D = NW_D * 2            # dest (window, parity) groups
RCH = 8                    # gather chunk columns (1024 idx <= carveout)

_programs = {}


def _build_program(key):
    import concourse.bass as bass
    import concourse.bacc as bacc
    import concourse.mybir as mybir
    import concourse.tile as tile

    GS = list(key[0])          # columns per source (window, parity) group
    GD = list(key[1])          # columns per dest (window, parity) group
    Gs, Gd = sum(GS), sum(GD)
    offs = np.concatenate([[0], np.cumsum(GS)]).astype(int)
    offd = np.concatenate([[0], np.cumsum(GD)]).astype(int)

    nc = bacc.Bacc("TRN2", target_bir_lowering=False, debug=False,
                   num_devices=NCORES)

    ref2 = nc.dram_tensor("ref2", [NPIX_H // 2, E128], mybir.dt.float32,
                          kind="ExternalInput")
    oth2 = nc.dram_tensor("oth2", [NPIX // 2, E128], mybir.dt.float32,
                          kind="ExternalInput")
    # idx streams (int16, 16-wrapped, replicated x8): [d | s | r]
    idx16 = nc.dram_tensor("idx16", [P, (Gd + 2 * Gs) * 8], mybir.dt.int16,
                           kind="ExternalInput")
    # valid masks: [pm_d | pm_s]
    pmio = nc.dram_tensor("pmio", [P, Gd + Gs], mybir.dt.float32,
                          kind="ExternalInput")
    scratch = nc.dram_tensor("scratch", [P * Gd, 64], mybir.dt.float32,
                             kind="Internal")
    out = nc.dram_tensor("out", [P, Gs + Gd], mybir.dt.float32,
                         kind="ExternalOutput")

    with tile.TileContext(nc) as tc:
        with (
            tc.tile_pool(name="gb", bufs=1) as gb,
            tc.tile_pool(name="cons", bufs=1) as cons,
        ):
            ix = gb.tile([P, (Gd + 2 * Gs) * 8], mybir.dt.int16)
            nc.sync.dma_start(out=ix[:], in_=idx16[:, :])
            pm = gb.tile([P, Gd + Gs], mybir.dt.float32)
            nc.sync.dma_start(out=pm[:], in_=pmio[:, :])

            OTH = gb.tile([P, Gd * E128], mybir.dt.float32)
            REF = gb.tile([P, Gs * E128], mybir.dt.float32)
            R2S = gb.tile([P, Gs * 64], mybir.dt.float32)

            def win_gathers(table, base8, offg, tile_out):
                """One dma_gather per (window, <=RCH column chunk); groups
                2w and 2w+1 belong to window w."""
                for wdx in range(len(offg) // 2):
                    lo, hi = int(offg[2 * wdx]), int(offg[2 * wdx + 2])
                    c0 = lo
                    while c0 < hi:
                        c1 = min(c0 + RCH, hi)
                        nc.gpsimd.dma_gather(
                            out_ap=tile_out[
                                :, c0 * E128:c1 * E128].rearrange(
                                "p (g c) -> p g c", c=E128),
                            in_ap=table[wdx * W_ROWS:(wdx + 1) * W_ROWS, :],
                            idxs_ap=ix[:, base8 + c0 * 8:base8 + c1 * 8],
                            num_idxs=(c1 - c0) * P,
                            num_idxs_reg=(c1 - c0) * P,
                            elem_size=E128,
                        )
                        c0 = c1

            def compact(src_tile, dst_tile, offg):
                """Per-(window, parity) strided copy of the valid 19
                channels into the packed [P, G*19] tile (parity selects
                the 64-slot half of the two-pixel row)."""
                for g in range(len(offg) - 1):
                    lo, hi = int(offg[g]), int(offg[g + 1])
                    if hi == lo:
                        continue
                    base = (g & 1) * 64
                    sv = src_tile[:, lo * E128:hi * E128].rearrange(
                        "p (g c) -> p g c", c=E128)[:, :, base:base + 19]
                    dv = dst_tile[:, lo * 19:hi * 19].rearrange(
                        "p (g c) -> p g c", c=19)
                    nc.vector.tensor_copy(out=dv, in_=sv)

            # d-side first: it feeds the compaction -> dump -> regather chain
            win_gathers(oth2, 0, offd, OTH)
            win_gathers(ref2, Gd * 8, offs, REF)

            # compact dest rows to packed [P, Gd*19]
            OPK = gb.tile([P, Gd * 19], mybir.dt.float32)
            compact(OTH, OPK, offd)
            OPKv = OPK[:].rearrange("p (g c) -> p g c", c=19)

            # dump packed dest rows to scratch row (p*Gd + g)
            nc.sync.dma_start(
                out=scratch.rearrange("(p g) c -> p g c", g=Gd)[:, :, 0:19],
                in_=OPKv,
            )

            # term2 in d-space: ln(sum_c exp(other_vec)), masked
            e2 = gb.tile([P, Gd * 19], mybir.dt.float32)
            e2v = e2[:].rearrange("p (g c) -> p g c", c=19)
            nc.scalar.activation(e2v, OPKv, mybir.ActivationFunctionType.Exp)
            S2 = gb.tile([P, Gd], mybir.dt.float32)
            nc.vector.tensor_reduce(out=S2[:], in_=e2v,
                                    axis=mybir.AxisListType.X,
                                    op=mybir.AluOpType.add)
            TG = cons.tile([P, Gs + Gd], mybir.dt.float32)
            L2 = TG[:, Gs:]
            nc.scalar.activation(L2, S2[:], mybir.ActivationFunctionType.Ln)
            nc.vector.tensor_tensor(out=L2, in0=L2, in1=pm[:, 0:Gd],
                                    op=mybir.AluOpType.mult)

            # s-space argmax one-hot (pm_s folded in)
            RPK = gb.tile([P, Gs * 19], mybir.dt.float32)
            compact(REF, RPK, offs)
            RPKv = RPK[:].rearrange("p (g c) -> p g c", c=19)
            m2 = gb.tile([P, Gs], mybir.dt.float32)
            nc.vector.tensor_reduce(out=m2[:], in_=RPKv,
                                    axis=mybir.AxisListType.X,
                                    op=mybir.AluOpType.max)
            eq2 = gb.tile([P, Gs * 19], mybir.dt.float32)
            eq2v = eq2[:].rearrange("p (g c) -> p g c", c=19)
            nc.vector.tensor_tensor(
                out=eq2v, in0=RPKv,
                in1=m2[:, :, None].to_broadcast([P, Gs, 19]),
                op=mybir.AluOpType.is_ge,
            )
            nc.vector.tensor_tensor(
                out=eq2v, in0=eq2v,
                in1=pm[:, Gd:, None].to_broadcast([P, Gs, 19]),
                op=mybir.AluOpType.mult,
            )

            # route other rows into s-slot order (chunked regather), pairing
            # each chunk as soon as it lands
            rbase = (Gd + Gs) * 8
            t1g = TG[:, 0:Gs]
            for lo in range(0, Gs, RCH):
                hi = min(lo + RCH, Gs)
                w = hi - lo
                nc.gpsimd.dma_gather(
                    out_ap=R2S[:, lo * 64:hi * 64].rearrange(
                        "p (g c) -> p g c", c=64),
                    in_ap=scratch[:, :],
                    idxs_ap=ix[:, rbase + lo * 8:rbase + hi * 8],
                    num_idxs=w * P,
                    num_idxs_reg=w * P,
                    elem_size=64,
                )
                R2v = R2S[:, lo * 64:hi * 64].rearrange(
                    "p (g c) -> p g c", c=64)[:, :, 0:19]
                eqc = eq2[:, lo * 19:hi * 19].rearrange(
                    "p (g c) -> p g c", c=19)
                nc.vector.tensor_tensor(out=eqc, in0=eqc, in1=R2v,
                                        op=mybir.AluOpType.mult)
                nc.vector.tensor_reduce(out=t1g[:, lo:hi], in_=eqc,
                                        axis=mybir.AxisListType.X,
                                        op=mybir.AluOpType.add)

            nc.sync.dma_start(out=out[:, :], in_=TG[:])

    nc.finalize()
    return nc


def _get_program(key):
    if key not in _programs:
        _programs[key] = _build_program(key)
    return _programs[key]


def _host_prep(inds_ref, inds_other):
    """Index-only host math: dedup scatter (last wins), split per core,
    sort both slot spaces by (two-pixel-row window, parity)."""
    ir = np.asarray(inds_ref).astype(np.int64)      # [B, 2, N]
    io = np.asarray(inds_other).astype(np.int64)
    valid = ((ir[:, 0] >= 0) & (ir[:, 0] < W) & (ir[:, 1] >= 0) & (ir[:, 1] < H)
             & (io[:, 0] >= 0) & (io[:, 0] < W) & (io[:, 1] >= 0)
             & (io[:, 1] < H))                       # [B, N]
    lin_ref = H * ir[:, 1] + ir[:, 0]                # [B, N]
    lin_other = H * io[:, 1] + io[:, 0]

    per_core = []
    count = 0
    for b in range(B):
        v = valid[b]
        lo = lin_other[b][v]
        lr = np.clip(lin_ref[b][v], 0, HW - 1)
        u, first_rev = np.unique(lo[::-1], return_index=True)
        d_arr = u.astype(np.int64)
        s_arr = lr[len(lo) - 1 - first_rev].astype(np.int64)
        count += len(u)
        for h in range(2):
            sel = (s_arr // NPIX_H) == h
            s_local = s_arr[sel] - h * NPIX_H
            d_sel = d_arr[sel]
            # group = (two-pixel-row window, pixel parity)
            gs = (s_local >> 16) * 2 + (s_local & 1)
            gd = (d_sel >> 16) * 2 + (d_sel & 1)
            s_ord = np.argsort(gs, kind='stable')
            d_ord = np.argsort(gd, kind='stable')
            per_core.append({
                "s": s_local[s_ord], "d": d_sel[d_ord],
                # for each s-sorted position, the d-sorted position of the
                # same correspondence (routing for the regather)
                "route": np.argsort(d_ord, kind='stable')[s_ord],
                "ngs": np.bincount(gs[s_ord], minlength=NG_S).astype(int),
                "ngd": np.bincount(gd[d_ord], minlength=NG_D).astype(int),
            })
    return per_core, count


def _plan(per_core):
    ngs = np.stack([pc["ngs"] for pc in per_core])
    ngd = np.stack([pc["ngd"] for pc in per_core])
    GS = np.maximum(1, -(-ngs.max(axis=0) // P))
    GD = np.maximum(1, -(-ngd.max(axis=0) // P))
    return (tuple(int(g) for g in GS), tuple(int(g) for g in GD))


def _wrap16(vals, ncols8):
    outp = np.zeros((16, ncols8), dtype=np.int16)
    j = np.arange(len(vals))
    outp[j % 16, j // 16] = vals.astype(np.int16)
    return outp


def _pack_core(pc, key):
    GS, GD = np.asarray(key[0]), np.asarray(key[1])
    Gs, Gd = int(GS.sum()), int(GD.sum())
    offs = np.concatenate([[0], np.cumsum(GS)]).astype(int)
    offd = np.concatenate([[0], np.cumsum(GD)]).astype(int)

    d_stream = np.zeros(Gd * P, dtype=np.int64)
    pm_d = np.zeros((P, Gd), dtype=np.float32)
    dpos2lin = np.zeros(len(pc["d"]), dtype=np.int64)
    pos = 0
    for g in range(NG_D):
        n = int(pc["ngd"][g])
        jj = np.arange(n)
        # window-local two-pixel row index
        d_stream[offd[g] * P + jj] = (pc["d"][pos:pos + n] >> 1) - \
            (g // 2) * W_ROWS
        gcol = offd[g] + jj // P
        pm_d[jj % P, gcol] = 1.0
        dpos2lin[pos:pos + n] = (jj % P) * Gd + gcol
        pos += n

    s_stream = np.zeros(Gs * P, dtype=np.int64)
    r_stream = np.zeros(Gs * P, dtype=np.int64)
    pm_s = np.zeros((P, Gs), dtype=np.float32)
    pos = 0
    for g in range(NG_S):
        n = int(pc["ngs"][g])
        jj = np.arange(n)
        s_stream[offs[g] * P + jj] = (pc["s"][pos:pos + n] >> 1) - \
            (g // 2) * W_ROWS
        r_stream[offs[g] * P + jj] = dpos2lin[pc["route"][pos:pos + n]]
        pm_s[jj % P, offs[g] + jj // P] = 1.0
        pos += n

    idx16 = np.concatenate([
        _wrap16(d_stream, Gd * 8),
        _wrap16(s_stream, Gs * 8),
        _wrap16(r_stream, Gs * 8),
    ], axis=1)
    idx16 = np.tile(idx16, (8, 1))
    pmio = np.concatenate([pm_d, pm_s], axis=1)
    return idx16, pmio


def _make_in_maps(inputs_ref, inputs_other, per_core, key):
    ref_flat = inputs_ref.reshape(B, C, HW)
    other_flat = inputs_other.reshape(B, C, HW)
    other_cache = {}
    in_maps = []
    for ci, pc in enumerate(per_core):
        b, h = ci // 2, ci % 2
        ref64 = np.empty((NPIX_H, 64), dtype=np.float32)
        ref64[:, :C] = ref_flat[b, :, h * NPIX_H:(h + 1) * NPIX_H].T
        if b not in other_cache:
            o64 = np.empty((NPIX, 64), dtype=np.float32)
            o64[:, :C] = other_flat[b, :, :NPIX].T
            other_cache[b] = o64.reshape(NPIX // 2, E128)
        idx16, pmio = _pack_core(pc, key)
        in_maps.append({
            "ref2": ref64.reshape(NPIX_H // 2, E128),
            "oth2": other_cache[b],
            "idx16": idx16,
            "pmio": pmio,
        })
    return in_maps


def kernel(inputs_ref, inputs_other, inds_ref, inds_other, weights):
    from concourse.bass_utils import run_bass_kernel_spmd

    inputs_ref = np.asarray(inputs_ref, dtype=np.float32)
    inputs_other = np.asarray(inputs_other, dtype=np.float32)

    per_core, count = _host_prep(inds_ref, inds_other)
    key = _plan(per_core)
    nc = _get_program(key)

    in_maps = _make_in_maps(inputs_ref, inputs_other, per_core, key)
    res = run_bass_kernel_spmd(nc, in_maps, core_ids=list(range(NCORES)))
    total = 0.0
    Gs = sum(key[0])
    for r in res.results:
        o = np.asarray(r["out"], dtype=np.float64)
        total += o[:, :Gs].sum() - o[:, Gs:].sum()
    loss = -total / max(count, 1)
    return np.float32(loss)


# revision 12
# speedup vs baseline: 3.4498x; 1.0305x over previous
"""Trainium2 Bass kernel for nn_CorrClassLoss.

Reference computation (B=4, C=19, H=512, W=1024, N=5000, IGNORE=255):
  ref_class = argmax_c inputs_ref[b].reshape(C, H*W)      # flat W-major
  lin_ref   = 512*y_ref + x_ref    (NOTE: linearized with H, kept faithfully)
  lin_other = 512*y_other + x_other
  gathered  = ref_class[b, lin_ref]
  target[b, lin_other] = gathered  (scatter, last write wins; rest IGNORE)
  loss = mean over non-ignored pixels of -log_softmax(inputs_other)[b, target, px]

Only flat positions [0, 262144) are touched; at most N unique scatter dests
per batch contribute:

  loss = -(1/cnt) * sum over unique dests d (last writer j, src s_j) of
         [ x_other[b, cls(s_j), d] - ln(sum_c exp(x_other[b, c, d])) ]
  cls(s) = argmax_c x_ref[b, c, s],  cnt = total unique dests.

Strategy (8 cores, data-parallel over (batch, half-of-sources)). Host does
index-only math (dedup last-wins, core split, window/parity sort, idx
packing) plus pure relayout of image data (pixel-major transpose into
64-slot rows; two pixels share one 512B table row so one int16-indexed
32K-row gather window covers 64K pixels). Device per core:
  - 4 (dest) + 4 (source) InstDMAGatherAnt fetch two-pixel rows into
    window/parity-sorted slot spaces (slot j = [j%128, j//128]); per-parity
    strided views compact the valid 19 channels into packed tiles.
  - dest side: ln(sum_c exp(.)) (masked) -> term2 partials; the packed
    dest rows are dumped to a DRAM scratch (one strided DMA) and regathered
    (3x InstDMAGatherAnt, chunked under the 1024-descriptor carveout) into
    source-slot order, where the argmax one-hot (max + is_ge) pairs them:
    term1 partials = sum one-hot . other_vec (mask folded into the one-hot).
  Output [128, Gs+Gd] = (term1 partials | term2 partials); host sums and
  computes loss = -(sum t1 - sum t2) / cnt.
"""

import sys

if "/opt/trn_rl_repo" not in sys.path:
    sys.path.insert(0, "/opt/trn_rl_repo")

import numpy as np

B, C, H, W = 4, 19, 512, 1024
HW = H * W                 # 524288
NPIX = 262144              # touched flat range [0, 262144)
NPIX_H = NPIX // 2         # 131072 source pixels per core
N = 5000
NCORES = 8

P = 128                    # partitions
E128 = 128                 # f32 slots per two-pixel table row (512B)
W_ROWS = 32768             # rows per dma_gather window (int16 idx range)
NW_S = NPIX_H // 2 // W_ROWS   # 2 source windows per core
NW_D = NPIX // 2 // W_ROWS     # 4 dest windows per core
NG_S = NW_S * 2            # source (window, parity) groups
NG_D = NW_D * 2            # dest (window, parity) groups
RCH = 8                    # gather chunk columns (1024 idx <= carveout)

_programs = {}


def _build_program(key):
    import concourse.bass as bass
    import concourse.bacc as bacc
    import concourse.mybir as mybir
    import concourse.tile as tile

    GS = list(key[0])          # columns per source (window, parity) group
    GD = list(key[1])          # columns per dest (window, parity) group
    Gs, Gd = sum(GS), sum(GD)
    offs = np.concatenate([[0], np.cumsum(GS)]).astype(int)
    offd = np.concatenate([[0], np.cumsum(GD)]).astype(int)

    nc = bacc.Bacc("TRN2", target_bir_lowering=False, debug=False,
                   num_devices=NCORES)

    ref2 = nc.dram_tensor("ref2", [NPIX_H // 2, E128], mybir.dt.float32,
                          kind="ExternalInput")
    oth2 = nc.dram_tensor("oth2", [NPIX // 2, E128], mybir.dt.float32,
                          kind="ExternalInput")
    # idx streams (int16, 16-wrapped, replicated x8): d gates the first
    # gathers so it uploads alone; s and r follow
    idx_d = nc.dram_tensor("idx_d", [P, Gd * 8], mybir.dt.int16,
                           kind="ExternalInput")
    idx_sr = nc.dram_tensor("idx_sr", [P, 2 * Gs * 8], mybir.dt.int16,
                            kind="ExternalInput")
    # valid masks: [pm_d | pm_s]
    pmio = nc.dram_tensor("pmio", [P, Gd + Gs], mybir.dt.float32,
                          kind="ExternalInput")
    scratch = nc.dram_tensor("scratch", [P * Gd, 64], mybir.dt.float32,
                             kind="Internal")
    out = nc.dram_tensor("out", [P, Gs + Gd], mybir.dt.float32,
                         kind="ExternalOutput")

    with tile.TileContext(nc) as tc:
        with (
            tc.tile_pool(name="gb", bufs=1) as gb,
            tc.tile_pool(name="cons", bufs=1) as cons,
        ):
            ixd = gb.tile([P, Gd * 8], mybir.dt.int16)
            nc.sync.dma_start(out=ixd[:], in_=idx_d[:, :])
            ixsr = gb.tile([P, 2 * Gs * 8], mybir.dt.int16)
            nc.sync.dma_start(out=ixsr[:], in_=idx_sr[:, :])
            pm = gb.tile([P, Gd + Gs], mybir.dt.float32)
            nc.sync.dma_start(out=pm[:], in_=pmio[:, :])

            OTH = gb.tile([P, Gd * E128], mybir.dt.float32)
            REF = gb.tile([P, Gs * E128], mybir.dt.float32)
            R2S = gb.tile([P, Gs * 64], mybir.dt.float32)

            def win_gathers(table, ixt, base8, offg, tile_out):
                """One dma_gather per (window, <=RCH column chunk); groups
                2w and 2w+1 belong to window w."""
                for wdx in range(len(offg) // 2):
                    lo, hi = int(offg[2 * wdx]), int(offg[2 * wdx + 2])
                    c0 = lo
                    while c0 < hi:
                        c1 = min(c0 + RCH, hi)
                        nc.gpsimd.dma_gather(
                            out_ap=tile_out[
                                :, c0 * E128:c1 * E128].rearrange(
                                "p (g c) -> p g c", c=E128),
                            in_ap=table[wdx * W_ROWS:(wdx + 1) * W_ROWS, :],
                            idxs_ap=ixt[:, base8 + c0 * 8:base8 + c1 * 8],
                            num_idxs=(c1 - c0) * P,
                            num_idxs_reg=(c1 - c0) * P,
                            elem_size=E128,
                        )
                        c0 = c1

            def compact(src_tile, dst_tile, offg, col0=None):
                """Per-(window, parity) strided copy of the valid 19
                channels into the packed [P, G*19] tile (parity selects
                the 64-slot half of the two-pixel row). offg holds absolute
                column offsets; group parity alternates from the parity of
                the first group's index, which is even for both slot spaces
                and for both dump halves (NG_D//2 is even)."""
                for g in range(len(offg) - 1):
                    lo, hi = int(offg[g]), int(offg[g + 1])
                    if hi == lo:
                        continue
                    base = (g & 1) * 64
                    sv = src_tile[:, lo * E128:hi * E128].rearrange(
                        "p (g c) -> p g c", c=E128)[:, :, base:base + 19]
                    dv = dst_tile[:, lo * 19:hi * 19].rearrange(
                        "p (g c) -> p g c", c=19)
                    nc.vector.tensor_copy(out=dv, in_=sv)

            # d-side first: it feeds the compaction -> dump -> regather chain
            win_gathers(oth2, ixd, 0, offd, OTH)
            win_gathers(ref2, ixsr, 0, offs, REF)

            # compact dest rows to packed [P, Gd*19]; dump each half of
            # the columns to scratch rows (p*Gd + g) as soon as compacted
            OPK = gb.tile([P, Gd * 19], mybir.dt.float32)
            OPKv = OPK[:].rearrange("p (g c) -> p g c", c=19)
            scr3 = scratch.rearrange("(p g) c -> p g c", g=Gd)
            mid = int(offd[NG_D // 2])
            compact(OTH, OPK, offd[:NG_D // 2 + 1])
            nc.sync.dma_start(out=scr3[:, 0:mid, 0:19],
                              in_=OPKv[:, 0:mid, :])
            compact(OTH, OPK, offd[NG_D // 2:], col0=mid)
            nc.sync.dma_start(out=scr3[:, mid:, 0:19],
                              in_=OPKv[:, mid:, :])

            # term2 in d-space: ln(sum_c exp(other_vec)), masked
            e2 = gb.tile([P, Gd * 19], mybir.dt.float32)
            e2v = e2[:].rearrange("p (g c) -> p g c", c=19)
            nc.scalar.activation(e2v, OPKv, mybir.ActivationFunctionType.Exp)
            S2 = gb.tile([P, Gd], mybir.dt.float32)
            nc.vector.tensor_reduce(out=S2[:], in_=e2v,
                                    axis=mybir.AxisListType.X,
                                    op=mybir.AluOpType.add)
            TG = cons.tile([P, Gs + Gd], mybir.dt.float32)
            L2 = TG[:, Gs:]
            nc.scalar.activation(L2, S2[:], mybir.ActivationFunctionType.Ln)
            nc.vector.tensor_tensor(out=L2, in0=L2, in1=pm[:, 0:Gd],
                                    op=mybir.AluOpType.mult)

            # s-space argmax one-hot (pm_s folded in)
            RPK = gb.tile([P, Gs * 19], mybir.dt.float32)
            compact(REF, RPK, offs)
            RPKv = RPK[:].rearrange("p (g c) -> p g c", c=19)
            m2 = gb.tile([P, Gs], mybir.dt.float32)
            nc.vector.tensor_reduce(out=m2[:], in_=RPKv,
                                    axis=mybir.AxisListType.X,
                                    op=mybir.AluOpType.max)
            eq2 = gb.tile([P, Gs * 19], mybir.dt.float32)
            eq2v = eq2[:].rearrange("p (g c) -> p g c", c=19)
            nc.vector.tensor_tensor(
                out=eq2v, in0=RPKv,
                in1=m2[:, :, None].to_broadcast([P, Gs, 19]),
                op=mybir.AluOpType.is_ge,
            )
            nc.vector.tensor_tensor(
                out=eq2v, in0=eq2v,
                in1=pm[:, Gd:, None].to_broadcast([P, Gs, 19]),
                op=mybir.AluOpType.mult,
            )

            # route other rows into s-slot order (chunked regather), pairing
            # each chunk as soon as it lands
            rbase = Gs * 8
            t1g = TG[:, 0:Gs]
            for lo in range(0, Gs, RCH):
                hi = min(lo + RCH, Gs)
                w = hi - lo
                nc.gpsimd.dma_gather(
                    out_ap=R2S[:, lo * 64:hi * 64].rearrange(
                        "p (g c) -> p g c", c=64),
                    in_ap=scratch[:, :],
                    idxs_ap=ixsr[:, rbase + lo * 8:rbase + hi * 8],
                    num_idxs=w * P,
                    num_idxs_reg=w * P,
                    elem_size=64,
                )
                R2v = R2S[:, lo * 64:hi * 64].rearrange(
                    "p (g c) -> p g c", c=64)[:, :, 0:19]
                eqc = eq2[:, lo * 19:hi * 19].rearrange(
                    "p (g c) -> p g c", c=19)
                nc.vector.tensor_tensor(out=eqc, in0=eqc, in1=R2v,
                                        op=mybir.AluOpType.mult)
                nc.vector.tensor_reduce(out=t1g[:, lo:hi], in_=eqc,
                                        axis=mybir.AxisListType.X,
                                        op=mybir.AluOpType.add)

            nc.sync.dma_start(out=out[:, :], in_=TG[:])

    nc.finalize()
    return nc


def _get_program(key):
    if key not in _programs:
        _programs[key] = _build_program(key)
    return _programs[key]


def _host_prep(inds_ref, inds_other):
    """Index-only host math: dedup scatter (last wins), split per core,
    sort both slot spaces by (two-pixel-row window, parity)."""
    ir = np.asarray(inds_ref).astype(np.int64)      # [B, 2, N]
    io = np.asarray(inds_other).astype(np.int64)
    valid = ((ir[:, 0] >= 0) & (ir[:, 0] < W) & (ir[:, 1] >= 0) & (ir[:, 1] < H)
             & (io[:, 0] >= 0) & (io[:, 0] < W) & (io[:, 1] >= 0)
             & (io[:, 1] < H))                       # [B, N]
    lin_ref = H * ir[:, 1] + ir[:, 0]                # [B, N]
    lin_other = H * io[:, 1] + io[:, 0]

    per_core = []
    count = 0
    for b in range(B):
        v = valid[b]
        lo = lin_other[b][v]
        lr = np.clip(lin_ref[b][v], 0, HW - 1)
        u, first_rev = np.unique(lo[::-1], return_index=True)
        d_arr = u.astype(np.int64)
        s_arr = lr[len(lo) - 1 - first_rev].astype(np.int64)
        count += len(u)
        for h in range(2):
            sel = (s_arr // NPIX_H) == h
            s_local = s_arr[sel] - h * NPIX_H
            d_sel = d_arr[sel]
            # group = (two-pixel-row window, pixel parity)
            gs = (s_local >> 16) * 2 + (s_local & 1)
            gd = (d_sel >> 16) * 2 + (d_sel & 1)
            s_ord = np.argsort(gs, kind='stable')
            d_ord = np.argsort(gd, kind='stable')
            per_core.append({
                "s": s_local[s_ord], "d": d_sel[d_ord],
                # for each s-sorted position, the d-sorted position of the
                # same correspondence (routing for the regather)
                "route": np.argsort(d_ord, kind='stable')[s_ord],
                "ngs": np.bincount(gs[s_ord], minlength=NG_S).astype(int),
                "ngd": np.bincount(gd[d_ord], minlength=NG_D).astype(int),
            })
    return per_core, count


def _plan(per_core):
    ngs = np.stack([pc["ngs"] for pc in per_core])
    ngd = np.stack([pc["ngd"] for pc in per_core])
    GS = np.maximum(1, -(-ngs.max(axis=0) // P))
    GD = np.maximum(1, -(-ngd.max(axis=0) // P))
    return (tuple(int(g) for g in GS), tuple(int(g) for g in GD))


def _wrap16(vals, ncols8):
    outp = np.zeros((16, ncols8), dtype=np.int16)
    j = np.arange(len(vals))
    outp[j % 16, j // 16] = vals.astype(np.int16)
    return outp


def _pack_core(pc, key):
    GS, GD = np.asarray(key[0]), np.asarray(key[1])
    Gs, Gd = int(GS.sum()), int(GD.sum())
    offs = np.concatenate([[0], np.cumsum(GS)]).astype(int)
    offd = np.concatenate([[0], np.cumsum(GD)]).astype(int)

    d_stream = np.zeros(Gd * P, dtype=np.int64)
    pm_d = np.zeros((P, Gd), dtype=np.float32)
    dpos2lin = np.zeros(len(pc["d"]), dtype=np.int64)
    pos = 0
    for g in range(NG_D):
        n = int(pc["ngd"][g])
        jj = np.arange(n)
        # window-local two-pixel row index
        d_stream[offd[g] * P + jj] = (pc["d"][pos:pos + n] >> 1) - \
            (g // 2) * W_ROWS
        gcol = offd[g] + jj // P
        pm_d[jj % P, gcol] = 1.0
        dpos2lin[pos:pos + n] = (jj % P) * Gd + gcol
        pos += n

    s_stream = np.zeros(Gs * P, dtype=np.int64)
    r_stream = np.zeros(Gs * P, dtype=np.int64)
    pm_s = np.zeros((P, Gs), dtype=np.float32)
    pos = 0
    for g in range(NG_S):
        n = int(pc["ngs"][g])
        jj = np.arange(n)
        s_stream[offs[g] * P + jj] = (pc["s"][pos:pos + n] >> 1) - \
            (g // 2) * W_ROWS
        r_stream[offs[g] * P + jj] = dpos2lin[pc["route"][pos:pos + n]]
        pm_s[jj % P, offs[g] + jj // P] = 1.0
        pos += n

    idx_d = np.tile(_wrap16(d_stream, Gd * 8), (8, 1))
    idx_sr = np.tile(np.concatenate([
        _wrap16(s_stream, Gs * 8),
        _wrap16(r_stream, Gs * 8),
    ], axis=1), (8, 1))
    pmio = np.concatenate([pm_d, pm_s], axis=1)
    return idx_d, idx_sr, pmio


def _make_in_maps(inputs_ref, inputs_other, per_core, key):
    ref_flat = inputs_ref.reshape(B, C, HW)
    other_flat = inputs_other.reshape(B, C, HW)
    other_cache = {}
    in_maps = []
    for ci, pc in enumerate(per_core):
        b, h = ci // 2, ci % 2
        ref64 = np.empty((NPIX_H, 64), dtype=np.float32)
        ref64[:, :C] = ref_flat[b, :, h * NPIX_H:(h + 1) * NPIX_H].T
        if b not in other_cache:
            o64 = np.empty((NPIX, 64), dtype=np.float32)
            o64[:, :C] = other_flat[b, :, :NPIX].T
            other_cache[b] = o64.reshape(NPIX // 2, E128)
        idx_d, idx_sr, pmio = _pack_core(pc, key)
        in_maps.append({
            "ref2": ref64.reshape(NPIX_H // 2, E128),
            "oth2": other_cache[b],
            "idx_d": idx_d,
            "idx_sr": idx_sr,
            "pmio": pmio,
        })
    return in_maps


def kernel(inputs_ref, inputs_other, inds_ref, inds_other, weights):
    from concourse.bass_utils import run_bass_kernel_spmd

    inputs_ref = np.asarray(inputs_ref, dtype=np.float32)
    inputs_other = np.asarray(inputs_other, dtype=np.float32)

    per_core, count = _host_prep(inds_ref, inds_other)
    key = _plan(per_core)
    nc = _get_program(key)

    in_maps = _make_in_maps(inputs_ref, inputs_other, per_core, key)
    res = run_bass_kernel_spmd(nc, in_maps, core_ids=list(range(NCORES)))
    total = 0.0
    Gs = sum(key[0])
    for r in res.results:
        o = np.asarray(r["out"], dtype=np.float64)
        total += o[:, :Gs].sum() - o[:, Gs:].sum()
    loss = -total / max(count, 1)
    return np.float32(loss)


# revision 13
# speedup vs baseline: 3.4730x; 1.0067x over previous
"""Trainium2 Bass kernel for nn_CorrClassLoss.

Reference computation (B=4, C=19, H=512, W=1024, N=5000, IGNORE=255):
  ref_class = argmax_c inputs_ref[b].reshape(C, H*W)      # flat W-major
  lin_ref   = 512*y_ref + x_ref    (NOTE: linearized with H, kept faithfully)
  lin_other = 512*y_other + x_other
  gathered  = ref_class[b, lin_ref]
  target[b, lin_other] = gathered  (scatter, last write wins; rest IGNORE)
  loss = mean over non-ignored pixels of -log_softmax(inputs_other)[b, target, px]

Only flat positions [0, 262144) are touched; at most N unique scatter dests
per batch contribute:

  loss = -(1/cnt) * sum over unique dests d (last writer j, src s_j) of
         [ x_other[b, cls(s_j), d] - ln(sum_c exp(x_other[b, c, d])) ]
  cls(s) = argmax_c x_ref[b, c, s],  cnt = total unique dests.

Strategy (8 cores, data-parallel over (batch, half-of-sources)). Host does
index-only math (dedup last-wins, core split, window/parity sort, idx
packing) plus pure relayout of image data (pixel-major transpose into
64-slot rows; two pixels share one 512B table row so one int16-indexed
32K-row gather window covers 64K pixels). Device per core:
  - 4 (dest) + 4 (source) InstDMAGatherAnt fetch two-pixel rows into
    window/parity-sorted slot spaces (slot j = [j%128, j//128]); per-parity
    strided views compact the valid 19 channels into packed tiles.
  - dest side: ln(sum_c exp(.)) (masked) -> term2 partials; the packed
    dest rows are dumped to a DRAM scratch (one strided DMA) and regathered
    (3x InstDMAGatherAnt, chunked under the 1024-descriptor carveout) into
    source-slot order, where the argmax one-hot (max + is_ge) pairs them:
    term1 partials = sum one-hot . other_vec (mask folded into the one-hot).
  Output [128, Gs+Gd] = (term1 partials | term2 partials); host sums and
  computes loss = -(sum t1 - sum t2) / cnt.
"""

import sys

if "/opt/trn_rl_repo" not in sys.path:
    sys.path.insert(0, "/opt/trn_rl_repo")

import numpy as np

B, C, H, W = 4, 19, 512, 1024
HW = H * W                 # 524288
NPIX = 262144              # touched flat range [0, 262144)
NPIX_H = NPIX // 2         # 131072 source pixels per core
N = 5000
NCORES = 8

P = 128                    # partitions
E128 = 128                 # f32 slots per two-pixel table row (512B)
W_ROWS = 32768             # rows per dma_gather window (int16 idx range)
NW_S = NPIX_H // 2 // W_ROWS   # 2 source windows per core
NW_D = NPIX // 2 // W_ROWS     # 4 dest windows per core
NG_S = NW_S * 2            # source (window, parity) groups
NG_D = NW_D * 2            # dest (window, parity) groups
RCH = 8                    # gather chunk columns (1024 idx <= carveout)

_programs = {}


def _build_program(key):
    import concourse.bass as bass
    import concourse.bacc as bacc
    import concourse.mybir as mybir
    import concourse.tile as tile

    GS = list(key[0])          # columns per source (window, parity) group
    GD = list(key[1])          # columns per dest (window, parity) group
    Gs, Gd = sum(GS), sum(GD)
    offs = np.concatenate([[0], np.cumsum(GS)]).astype(int)
    offd = np.concatenate([[0], np.cumsum(GD)]).astype(int)

    nc = bacc.Bacc("TRN2", target_bir_lowering=False, debug=False,
                   num_devices=NCORES)

    ref2 = nc.dram_tensor("ref2", [NPIX_H // 2, E128], mybir.dt.float32,
                          kind="ExternalInput")
    oth2 = nc.dram_tensor("oth2", [NPIX // 2, E128], mybir.dt.float32,
                          kind="ExternalInput")
    # idx streams (int16, 16-wrapped, replicated x8): d gates the first
    # gathers so it uploads alone; s and r follow
    idx_d = nc.dram_tensor("idx_d", [P, Gd * 8], mybir.dt.int16,
                           kind="ExternalInput")
    idx_sr = nc.dram_tensor("idx_sr", [P, 2 * Gs * 8], mybir.dt.int16,
                            kind="ExternalInput")
    # valid masks: [pm_d | pm_s]
    pmio = nc.dram_tensor("pmio", [P, Gd + Gs], mybir.dt.float32,
                          kind="ExternalInput")
    scratch = nc.dram_tensor("scratch", [P * Gd, 64], mybir.dt.float32,
                             kind="Internal")
    out = nc.dram_tensor("out", [P, Gs + Gd], mybir.dt.float32,
                         kind="ExternalOutput")

    with tile.TileContext(nc) as tc:
        with (
            tc.tile_pool(name="gb", bufs=1) as gb,
            tc.tile_pool(name="cons", bufs=1) as cons,
        ):
            ixd = gb.tile([P, Gd * 8], mybir.dt.int16)
            nc.sync.dma_start(out=ixd[:], in_=idx_d[:, :])
            ixsr = gb.tile([P, 2 * Gs * 8], mybir.dt.int16)
            nc.sync.dma_start(out=ixsr[:], in_=idx_sr[:, :])
            pm = gb.tile([P, Gd + Gs], mybir.dt.float32)
            nc.sync.dma_start(out=pm[:], in_=pmio[:, :])

            OTH = gb.tile([P, Gd * E128], mybir.dt.float32)
            REF = gb.tile([P, Gs * E128], mybir.dt.float32)
            R2S = gb.tile([P, Gs * 64], mybir.dt.float32)

            def win_gathers(table, ixt, base8, offg, tile_out):
                """One dma_gather per (window, <=RCH column chunk); groups
                2w and 2w+1 belong to window w."""
                for wdx in range(len(offg) // 2):
                    lo, hi = int(offg[2 * wdx]), int(offg[2 * wdx + 2])
                    c0 = lo
                    while c0 < hi:
                        c1 = min(c0 + RCH, hi)
                        nc.gpsimd.dma_gather(
                            out_ap=tile_out[
                                :, c0 * E128:c1 * E128].rearrange(
                                "p (g c) -> p g c", c=E128),
                            in_ap=table[wdx * W_ROWS:(wdx + 1) * W_ROWS, :],
                            idxs_ap=ixt[:, base8 + c0 * 8:base8 + c1 * 8],
                            num_idxs=(c1 - c0) * P,
                            num_idxs_reg=(c1 - c0) * P,
                            elem_size=E128,
                        )
                        c0 = c1

            def compact(src_tile, dst_tile, offg, col0=None):
                """Per-(window, parity) strided copy of the valid 19
                channels into the packed [P, G*19] tile (parity selects
                the 64-slot half of the two-pixel row). offg holds absolute
                column offsets; group parity alternates from the parity of
                the first group's index, which is even for both slot spaces
                and for both dump halves (NG_D//2 is even)."""
                for g in range(len(offg) - 1):
                    lo, hi = int(offg[g]), int(offg[g + 1])
                    if hi == lo:
                        continue
                    base = (g & 1) * 64
                    sv = src_tile[:, lo * E128:hi * E128].rearrange(
                        "p (g c) -> p g c", c=E128)[:, :, base:base + 19]
                    dv = dst_tile[:, lo * 19:hi * 19].rearrange(
                        "p (g c) -> p g c", c=19)
                    nc.vector.tensor_copy(out=dv, in_=sv)

            # d-side first: it feeds the compaction -> dump -> regather chain
            win_gathers(oth2, ixd, 0, offd, OTH)
            win_gathers(ref2, ixsr, 0, offs, REF)

            # compact dest rows to packed [P, Gd*19]; dump each dest
            # window's columns to scratch rows (p*Gd + g) as soon as its
            # gather lands, so the last (smallest) dump gates the regather
            # as briefly as possible
            OPK = gb.tile([P, Gd * 19], mybir.dt.float32)
            OPKv = OPK[:].rearrange("p (g c) -> p g c", c=19)
            scr3 = scratch.rearrange("(p g) c -> p g c", g=Gd)
            for m in range(NW_D):
                lo, hi = int(offd[2 * m]), int(offd[2 * m + 2])
                if hi == lo:
                    continue
                compact(OTH, OPK, offd[2 * m:2 * m + 3])
                nc.sync.dma_start(out=scr3[:, lo:hi, 0:19],
                                  in_=OPKv[:, lo:hi, :])

            # term2 in d-space: ln(sum_c exp(other_vec)), masked
            e2 = gb.tile([P, Gd * 19], mybir.dt.float32)
            e2v = e2[:].rearrange("p (g c) -> p g c", c=19)
            nc.scalar.activation(e2v, OPKv, mybir.ActivationFunctionType.Exp)
            S2 = gb.tile([P, Gd], mybir.dt.float32)
            nc.vector.tensor_reduce(out=S2[:], in_=e2v,
                                    axis=mybir.AxisListType.X,
                                    op=mybir.AluOpType.add)
            TG = cons.tile([P, Gs + Gd], mybir.dt.float32)
            L2 = TG[:, Gs:]
            nc.scalar.activation(L2, S2[:], mybir.ActivationFunctionType.Ln)
            nc.vector.tensor_tensor(out=L2, in0=L2, in1=pm[:, 0:Gd],
                                    op=mybir.AluOpType.mult)

            # s-space argmax one-hot (pm_s folded in)
            RPK = gb.tile([P, Gs * 19], mybir.dt.float32)
            compact(REF, RPK, offs)
            RPKv = RPK[:].rearrange("p (g c) -> p g c", c=19)
            m2 = gb.tile([P, Gs], mybir.dt.float32)
            nc.vector.tensor_reduce(out=m2[:], in_=RPKv,
                                    axis=mybir.AxisListType.X,
                                    op=mybir.AluOpType.max)
            eq2 = gb.tile([P, Gs * 19], mybir.dt.float32)
            eq2v = eq2[:].rearrange("p (g c) -> p g c", c=19)
            nc.vector.tensor_tensor(
                out=eq2v, in0=RPKv,
                in1=m2[:, :, None].to_broadcast([P, Gs, 19]),
                op=mybir.AluOpType.is_ge,
            )
            nc.vector.tensor_tensor(
                out=eq2v, in0=eq2v,
                in1=pm[:, Gd:, None].to_broadcast([P, Gs, 19]),
                op=mybir.AluOpType.mult,
            )

            # route other rows into s-slot order (chunked regather), pairing
            # each chunk as soon as it lands
            rbase = Gs * 8
            t1g = TG[:, 0:Gs]
            for lo in range(0, Gs, RCH):
                hi = min(lo + RCH, Gs)
                w = hi - lo
                nc.gpsimd.dma_gather(
                    out_ap=R2S[:, lo * 64:hi * 64].rearrange(
                        "p (g c) -> p g c", c=64),
                    in_ap=scratch[:, :],
                    idxs_ap=ixsr[:, rbase + lo * 8:rbase + hi * 8],
                    num_idxs=w * P,
                    num_idxs_reg=w * P,
                    elem_size=64,
                )
                R2v = R2S[:, lo * 64:hi * 64].rearrange(
                    "p (g c) -> p g c", c=64)[:, :, 0:19]
                eqc = eq2[:, lo * 19:hi * 19].rearrange(
                    "p (g c) -> p g c", c=19)
                nc.vector.tensor_tensor(out=eqc, in0=eqc, in1=R2v,
                                        op=mybir.AluOpType.mult)
                nc.vector.tensor_reduce(out=t1g[:, lo:hi], in_=eqc,
                                        axis=mybir.AxisListType.X,
                                        op=mybir.AluOpType.add)

            nc.sync.dma_start(out=out[:, :], in_=TG[:])

    nc.finalize()
    return nc


def _get_program(key):
    if key not in _programs:
        _programs[key] = _build_program(key)
    return _programs[key]


def _host_prep(inds_ref, inds_other):
    """Index-only host math: dedup scatter (last wins), split per core,
    sort both slot spaces by (two-pixel-row window, parity)."""
    ir = np.asarray(inds_ref).astype(np.int64)      # [B, 2, N]
    io = np.asarray(inds_other).astype(np.int64)
    valid = ((ir[:, 0] >= 0) & (ir[:, 0] < W) & (ir[:, 1] >= 0) & (ir[:, 1] < H)
             & (io[:, 0] >= 0) & (io[:, 0] < W) & (io[:, 1] >= 0)
             & (io[:, 1] < H))                       # [B, N]
    lin_ref = H * ir[:, 1] + ir[:, 0]                # [B, N]
    lin_other = H * io[:, 1] + io[:, 0]

    per_core = []
    count = 0
    for b in range(B):
        v = valid[b]
        lo = lin_other[b][v]
        lr = np.clip(lin_ref[b][v], 0, HW - 1)
        u, first_rev = np.unique(lo[::-1], return_index=True)
        d_arr = u.astype(np.int64)
        s_arr = lr[len(lo) - 1 - first_rev].astype(np.int64)
        count += len(u)
        for h in range(2):
            sel = (s_arr // NPIX_H) == h
            s_local = s_arr[sel] - h * NPIX_H
            d_sel = d_arr[sel]
            # group = (two-pixel-row window, pixel parity)
            gs = (s_local >> 16) * 2 + (s_local & 1)
            gd = (d_sel >> 16) * 2 + (d_sel & 1)
            s_ord = np.argsort(gs, kind='stable')
            d_ord = np.argsort(gd, kind='stable')
            per_core.append({
                "s": s_local[s_ord], "d": d_sel[d_ord],
                # for each s-sorted position, the d-sorted position of the
                # same correspondence (routing for the regather)
                "route": np.argsort(d_ord, kind='stable')[s_ord],
                "ngs": np.bincount(gs[s_ord], minlength=NG_S).astype(int),
                "ngd": np.bincount(gd[d_ord], minlength=NG_D).astype(int),
            })
    return per_core, count


def _plan(per_core):
    ngs = np.stack([pc["ngs"] for pc in per_core])
    ngd = np.stack([pc["ngd"] for pc in per_core])
    GS = np.maximum(1, -(-ngs.max(axis=0) // P))
    GD = np.maximum(1, -(-ngd.max(axis=0) // P))
    return (tuple(int(g) for g in GS), tuple(int(g) for g in GD))


def _wrap16(vals, ncols8):
    outp = np.zeros((16, ncols8), dtype=np.int16)
    j = np.arange(len(vals))
    outp[j % 16, j // 16] = vals.astype(np.int16)
    return outp


def _pack_core(pc, key):
    GS, GD = np.asarray(key[0]), np.asarray(key[1])
    Gs, Gd = int(GS.sum()), int(GD.sum())
    offs = np.concatenate([[0], np.cumsum(GS)]).astype(int)
    offd = np.concatenate([[0], np.cumsum(GD)]).astype(int)

    d_stream = np.zeros(Gd * P, dtype=np.int64)
    pm_d = np.zeros((P, Gd), dtype=np.float32)
    dpos2lin = np.zeros(len(pc["d"]), dtype=np.int64)
    pos = 0
    for g in range(NG_D):
        n = int(pc["ngd"][g])
        jj = np.arange(n)
        # window-local two-pixel row index
        d_stream[offd[g] * P + jj] = (pc["d"][pos:pos + n] >> 1) - \
            (g // 2) * W_ROWS
        gcol = offd[g] + jj // P
        pm_d[jj % P, gcol] = 1.0
        dpos2lin[pos:pos + n] = (jj % P) * Gd + gcol
        pos += n

    s_stream = np.zeros(Gs * P, dtype=np.int64)
    r_stream = np.zeros(Gs * P, dtype=np.int64)
    pm_s = np.zeros((P, Gs), dtype=np.float32)
    pos = 0
    for g in range(NG_S):
        n = int(pc["ngs"][g])
        jj = np.arange(n)
        s_stream[offs[g] * P + jj] = (pc["s"][pos:pos + n] >> 1) - \
            (g // 2) * W_ROWS
        r_stream[offs[g] * P + jj] = dpos2lin[pc["route"][pos:pos + n]]
        pm_s[jj % P, offs[g] + jj // P] = 1.0
        pos += n

    idx_d = np.tile(_wrap16(d_stream, Gd * 8), (8, 1))
    idx_sr = np.tile(np.concatenate([
        _wrap16(s_stream, Gs * 8),
        _wrap16(r_stream, Gs * 8),
    ], axis=1), (8, 1))
    pmio = np.concatenate([pm_d, pm_s], axis=1)
    return idx_d, idx_sr, pmio


def _make_in_maps(inputs_ref, inputs_other, per_core, key):
    ref_flat = inputs_ref.reshape(B, C, HW)
    other_flat = inputs_other.reshape(B, C, HW)
    other_cache = {}
    in_maps = []
    for ci, pc in enumerate(per_core):
        b, h = ci // 2, ci % 2
        ref64 = np.empty((NPIX_H, 64), dtype=np.float32)
        ref64[:, :C] = ref_flat[b, :, h * NPIX_H:(h + 1) * NPIX_H].T
        if b not in other_cache:
            o64 = np.empty((NPIX, 64), dtype=np.float32)
            o64[:, :C] = other_flat[b, :, :NPIX].T
            other_cache[b] = o64.reshape(NPIX // 2, E128)
        idx_d, idx_sr, pmio = _pack_core(pc, key)
        in_maps.append({
            "ref2": ref64.reshape(NPIX_H // 2, E128),
            "oth2": other_cache[b],
            "idx_d": idx_d,
            "idx_sr": idx_sr,
            "pmio": pmio,
        })
    return in_maps


def kernel(inputs_ref, inputs_other, inds_ref, inds_other, weights):
    from concourse.bass_utils import run_bass_kernel_spmd

    inputs_ref = np.asarray(inputs_ref, dtype=np.float32)
    inputs_other = np.asarray(inputs_other, dtype=np.float32)

    per_core, count = _host_prep(inds_ref, inds_other)
    key = _plan(per_core)
    nc = _get_program(key)

    in_maps = _make_in_maps(inputs_ref, inputs_other, per_core, key)
    res = run_bass_kernel_spmd(nc, in_maps, core_ids=list(range(NCORES)))
    total = 0.0
    Gs = sum(key[0])
    for r in res.results:
        o = np.asarray(r["out"], dtype=np.float64)
        total += o[:, :Gs].sum() - o[:, Gs:].sum()
    loss = -total / max(count, 1)
    return np.float32(loss)
